# revision 28
# baseline (speedup 1.0000x reference)
"""Hetero GNN encoder/decoder (SAGE x2 + BN + edge MLP decoder) on 8 trn2 cores.

v2 strategy (edge sharding by destination, node-range sharding):
  - Articles: core k owns rows [k*APC, (k+1)*APC); customers likewise (CPC,
    split in two halves per core).
  - All one-hot scatter matrices ("P strips") are STATIC edge structure and
    are precomputed on the host and shipped as inputs (fp16, 1/cnt scale
    folded in).  No DVE one-hot builds on device at all.
  - Layer 1 needs x_customer/x_article rows in edge order: these are also a
    static permutation of the inputs, so the host ships packed X streams and
    layer 1 runs with ZERO runtime gathers: stream X+P from DRAM, matmul
    into PSUM superblocks (feat on partitions, dst nodes on columns).
  - Layer 2 gathers h rows at runtime via one dma_gather per (sb, src-block)
    run (big calls amortize the ~1us SWDGE fixed overhead), then the same
    shipped-P matmul scatter.
  - hc is AllGathered in two halves (half-major relabeled row space) so the
    second AG overlaps the first half of layer-2 work.
  - Decoder uses precomputed U_c/U_a tables; per label
    y = w2 . relu(U_c[lc]+U_a[la]) + b2 with 4096-label gather chunks; relu
    on the scalar engine.

All structure (loop bounds, emissions) is compile-time and identical across
cores; per-core variation lives in the data (padded to uniform sizes).
"""
import sys

sys.path.insert(0, "/opt/trn_rl_repo")

import ml_dtypes
import numpy as np

import concourse.bacc as bacc
import concourse.bass as bass
import concourse.mybir as mybir
import concourse.tile as tile
from concourse.bass_utils import run_bass_kernel_spmd
from concourse.masks import make_identity

P = 128
NCORES = 8
BN_EPS = 1e-5
SIM_SINGLE_QUEUE = False   # set True when validating under CoreSim
DGCH = 4096           # decoder gather chunk
XCH = 8192            # L1 X/P stream load chunk (columns)
BANK = 512            # psum fp32 cols per bank


class Cfg:
    def __init__(self, n_c=300000, n_a=100000, e_lbl=1000000,
                 sbn=1024, w1a=16, w1c=16, w2a=32, w2c=32,
                 srcb_c=30000, srcb_a=25000):
        self.n_c, self.n_a, self.e_lbl = n_c, n_a, e_lbl
        self.cpc, self.apc = n_c // NCORES, n_a // NCORES
        assert self.cpc * NCORES == n_c and self.apc * NCORES == n_a
        self.chalf = self.cpc // 2
        self.sbn = sbn
        assert sbn % BANK == 0
        self.w1a, self.w1c, self.w2a, self.w2c = w1a, w1c, w2a, w2c
        self.srcb_c, self.srcb_a = srcb_c, srcb_a
        self.nblk_c = -(-n_c // srcb_c)
        self.nblk_a = -(-n_a // srcb_a)
        assert srcb_c < 32768 and srcb_a < 32768
        assert (n_c // 2) % srcb_c == 0   # j-blocks must not straddle halves
        self.zc_sub = self.chalf
        self.dt = mybir.dt.float16
        self.npdt = np.float16
        self.dt8 = mybir.dt.float8e4
        self.np8 = ml_dtypes.float8_e4m3


def _ru(x, m):
    return (x + m - 1) // m * m


def _wrap_idx(flat):
    """[n] int16 -> [128, n/16] wrap (16-partition, replicated x8)."""
    n = flat.shape[0]
    w = flat.astype(np.int16).reshape(n // 16, 16).T
    return np.tile(w, (8, 1))


# ---------------------------------------------------------------------------
# host-side structure + array prep
# ---------------------------------------------------------------------------

def _emissions(dstrel_all, T, W, nwin):
    """Per-tile strip ranges (union over cores).  dstrel_all: [NC, T*128]
    with -1e9 for pad.  Returns list of (t, wlo, nw)."""
    Dw = dstrel_all.reshape(NCORES, T, P)
    valid = Dw.max(axis=2) >= 0                    # [NC, T]
    wlo_c = np.where(Dw >= 0, Dw, np.inf).min(axis=2) // W
    whi_c = np.where(Dw >= 0, Dw, -np.inf).max(axis=2) // W
    any_v = valid.any(axis=0)
    wlo = np.where(valid, wlo_c, np.inf).min(axis=0)
    whi = np.where(valid, whi_c, -np.inf).max(axis=0)
    out = []
    for t in range(T):
        if not any_v[t]:
            continue
        a = max(0, min(int(wlo[t]), nwin - 1))
        b = max(a, min(int(whi[t]), nwin - 1))
        out.append((t, a, b - a + 1))
    return out


def _build_p(dstrel, scl, ems, T, W):
    """P stream [128, ncols] fp16 for one core.  dstrel/scl: [T*128]."""
    if not ems:
        return np.zeros((P, W), np.float16)
    et = np.array([e[0] for e in ems for _ in range(e[2])])
    ew = np.array([e[1] + i for e in ems for i in range(e[2])])
    D = dstrel.reshape(T, P)[et]                     # [ne, 128]
    S = scl.reshape(T, P)[et]
    del S
    tgt = (ew[:, None] * W + np.arange(W)[None, :])  # [ne, W]
    blk = (D[:, :, None] == tgt[:, None, :])         # binary (scale applied
    return np.ascontiguousarray(                     # post-agg on device)
        blk.transpose(1, 0, 2).reshape(P, -1)).astype(np.float16)


class L1Pass:
    """Layer-1 aggregation: host-packed X stream + P strips, no gathers."""

    def __init__(self, name, nloc, sbn, W):
        self.name, self.nloc, self.sbn, self.W = name, nloc, sbn, W
        self.nsb = -(-nloc // sbn)
        self.sb_nodes = [min(sbn, nloc - s * sbn) for s in range(self.nsb)]
        self.sb_len = None       # [nsb] padded slots per superblock
        self.offs = None
        self.etot = 0
        self.emits = None        # per sb: list of (t_local, wlo, nw)
        self.p_off = None        # per sb: P-stream col offset of first emit
        self.p_cols = 0
        self.src_order = None    # per core [etot] int64 (for X pack)
        self.pstreams = None     # per core [128, p_cols] fp16


def prep_l1(name, src, dst_loc, scl_e, core_e, nloc, sbn, W):
    ap = L1Pass(name, nloc, sbn, W)
    nsb = ap.nsb
    per_core = []
    counts = np.zeros((NCORES, nsb), np.int64)
    for k in range(NCORES):
        m = core_e == k
        s, d, sc = src[m], dst_loc[m], scl_e[m]
        order = np.argsort(d, kind="stable")
        s, d, sc = s[order], d[order], sc[order]
        sb = d // sbn
        counts[k] = np.bincount(sb, minlength=nsb)
        per_core.append((s, d, sc, sb))
    sb_len = _ru(counts.max(axis=0), P)
    offs = np.concatenate([[0], np.cumsum(sb_len)]).astype(np.int64)
    etot = int(offs[-1])
    ap.sb_len, ap.offs, ap.etot = sb_len, offs, etot

    dstrel_all = np.full((NCORES, etot), -1.0e9, np.float64)
    src_order, scl_all = [], []
    for k in range(NCORES):
        s, d, sc, sb = per_core[k]
        start = np.concatenate([[0], np.cumsum(counts[k])])[:-1]
        pos = offs[sb] + (np.arange(len(s)) - start[sb])
        so = np.zeros(etot, np.int64)
        so[pos] = s
        sl = np.zeros(etot, np.float32)
        sl[pos] = sc
        dstrel_all[k, pos] = (d - sb * sbn).astype(np.float64)
        src_order.append(so)
        scl_all.append(sl)
    ap.src_order = src_order

    emits, p_off = [], []
    pc = 0
    for s in range(nsb):
        T = int(sb_len[s]) // P
        nwin = -(-ap.sb_nodes[s] // W)
        o = int(offs[s])
        ems = _emissions(dstrel_all[:, o:o + sb_len[s]], T, W, nwin)
        emits.append(ems)
        p_off.append(pc)
        pc += sum(e[2] for e in ems) * W
    pc = _ru(max(pc, W), XCH)
    ap.emits, ap.p_off, ap.p_cols = emits, p_off, pc

    pstreams = []
    for k in range(NCORES):
        parts = []
        for s in range(nsb):
            if not emits[s]:
                continue
            o = int(offs[s])
            L = int(sb_len[s])
            dr = np.where(dstrel_all[k, o:o + L] < -1.0, -1000.0,
                          dstrel_all[k, o:o + L])
            parts.append(_build_p(dr, scl_all[k][o:o + L], emits[s],
                                  L // P, W))
        full = np.zeros((P, pc), np.float16)
        if parts:
            ps = np.concatenate(parts, axis=1)
            full[:, :ps.shape[1]] = ps
        pstreams.append(full)
    ap.pstreams = pstreams
    return ap


class L2Pass:
    """Layer-2 aggregation: runtime gathers per (sb, src-block) run +
    shipped P strips."""

    def __init__(self, name, nloc, srcb, nblk, nsrc_rows, sbn, W):
        self.name, self.nloc, self.srcb, self.nblk = name, nloc, srcb, nblk
        self.nsrc_rows, self.sbn, self.W = nsrc_rows, sbn, W
        self.nsb = -(-nloc // sbn)
        self.sb_nodes = [min(sbn, nloc - s * sbn) for s in range(self.nsb)]
        self.run_L = None        # [nsb, nblk]
        self.offs = None
        self.etot = 0
        self.emits = None        # per sb: list of (j, t_run, wlo, nw)
        self.p_off = None
        self.p_cols = 0
        self.idx = None          # per core [128, etot/16] int16
        self.pstreams = None


def prep_l2(name, src, dst_loc, scl_e, core_e, nloc, srcb, nblk,
            nsrc_rows, sbn, W):
    ap = L2Pass(name, nloc, srcb, nblk, nsrc_rows, sbn, W)
    nsb = ap.nsb
    nruns = nsb * nblk
    per_core = []
    counts = np.zeros((NCORES, nruns), np.int64)
    for k in range(NCORES):
        m = core_e == k
        s, d, sc = src[m], dst_loc[m], scl_e[m]
        j = s // srcb
        sb = d // sbn
        order = np.lexsort((d, j, sb))
        s, d, sc, j, sb = s[order], d[order], sc[order], j[order], sb[order]
        rid = sb * nblk + j
        counts[k] = np.bincount(rid, minlength=nruns)
        per_core.append((s, d, sc, rid, sb))
    run_L = _ru(counts.max(axis=0), P)
    offs = np.concatenate([[0], np.cumsum(run_L)]).astype(np.int64)
    etot = int(offs[-1])
    ap.run_L, ap.offs, ap.etot = run_L.reshape(nsb, nblk), offs, etot

    dstrel_all = np.full((NCORES, etot), -1.0e9, np.float64)
    idxs, scl_all = [], []
    for k in range(NCORES):
        s, d, sc, rid, sb = per_core[k]
        start = np.concatenate([[0], np.cumsum(counts[k])])[:-1]
        pos = offs[rid] + (np.arange(len(s)) - start[rid])
        i16 = np.zeros(etot, np.int16)
        i16[pos] = (s - (s // srcb) * srcb).astype(np.int16)
        sl = np.zeros(etot, np.float32)
        sl[pos] = sc
        dstrel_all[k, pos] = (d - sb * sbn).astype(np.float64)
        idxs.append(_wrap_idx(i16))
        scl_all.append(sl)
    ap.idx = idxs

    emits, p_off = [], []
    pc = 0
    for s in range(nsb):
        nwin = -(-ap.sb_nodes[s] // W)
        sb_ems = []
        for j in range(nblk):
            r = s * nblk + j
            o = int(offs[r])
            L = int(ap.run_L[s, j])
            if L == 0:
                continue
            ems = _emissions(dstrel_all[:, o:o + L], L // P, W, nwin)
            sb_ems.extend((j, t, a, nw) for (t, a, nw) in ems)
        emits.append(sb_ems)
        p_off.append(pc)
        pc += sum(e[3] for e in sb_ems) * W
    pc = _ru(max(pc, W), XCH)
    ap.emits, ap.p_off, ap.p_cols = emits, p_off, pc

    # flatten all strips (pass order) for vectorized P build
    em_base, em_w = [], []
    for s in range(nsb):
        for (j, t, a, nw) in emits[s]:
            o = int(offs[s * nblk + j]) + t * P
            for wi in range(nw):
                em_base.append(o)
                em_w.append(a + wi)
    em_base = np.array(em_base, np.int64)
    em_w = np.array(em_w, np.int64)
    pstreams = []
    for k in range(NCORES):
        if len(em_base) == 0:
            pstreams.append(np.zeros((P, pc), np.float16))
            continue
        D = dstrel_all[k][em_base[:, None] + np.arange(P)[None, :]]
        D = np.where(D < -1.0, -1000.0, D)
        tgt = em_w[:, None] * W + np.arange(W)[None, :]
        blk = (D[:, :, None] == tgt[:, None, :])
        ps = blk.transpose(1, 0, 2).reshape(P, -1)
        full = np.zeros((P, pc), np.float16)
        full[:, :ps.shape[1]] = ps.astype(np.float16)
        pstreams.append(full)
    ap.pstreams = pstreams
    return ap


def prep_all(cfg, inputs):
    i64 = lambda a: np.asarray(a).astype(np.int64)
    e_src = i64(inputs["edge_src_customer"])
    e_dst = i64(inputs["edge_dst_article"])
    l_c = i64(inputs["label_customer"])
    l_a = i64(inputs["label_article"])

    cnt_a = np.bincount(e_dst, minlength=cfg.n_a)
    cnt_c = np.bincount(e_src, minlength=cfg.n_c)
    scl_a = (1.0 / np.maximum(cnt_a, 1.0)).astype(np.float32)
    scl_c = (1.0 / np.maximum(cnt_c, 1.0)).astype(np.float32)

    core_a = e_dst // cfg.apc          # A-pass owner (dst article)
    core_c = e_src // cfg.cpc          # C-pass owner (dst customer)
    dloc_c = e_src % cfg.cpc
    half = (dloc_c >= cfg.chalf).astype(np.int64)

    # ---- layer 1 (host-packed X, no gathers) ----
    pa1 = prep_l1("A1", e_src, e_dst % cfg.apc, scl_a[e_dst], core_a,
                  cfg.apc, cfg.sbn, cfg.w1a)
    pc1 = []
    for h in range(2):
        m = half == h
        pc1.append(prep_l1(f"C1{h}", e_dst[m], dloc_c[m] - h * cfg.chalf,
                           scl_c[e_src][m], core_c[m], cfg.chalf, cfg.sbn,
                           cfg.w1c))

    # ---- layer 2 (runtime gathers; customers relabeled half-major) ----
    rel_c = half * (cfg.n_c // 2) + (e_src // cfg.cpc) * cfg.chalf + \
        (dloc_c - half * cfg.chalf)
    pa2 = prep_l2("A2", rel_c, e_dst % cfg.apc, scl_a[e_dst], core_a,
                  cfg.apc, cfg.srcb_c, cfg.nblk_c, cfg.n_c, cfg.sbn,
                  cfg.w2a)
    pc2 = []
    for h in range(2):
        m = half == h
        pc2.append(prep_l2(f"C2{h}", e_dst[m], dloc_c[m] - h * cfg.chalf,
                           scl_c[e_src][m], core_c[m], cfg.chalf,
                           cfg.srcb_a, cfg.nblk_a, cfg.n_a, cfg.sbn,
                           cfg.w2c))

    # ---- decoder labels: partition by customer core, group (sub, ablk) ----
    core_l = l_c // cfg.cpc
    sub_l = (l_c % cfg.cpc) // cfg.zc_sub
    ablk_l = l_a // cfg.srcb_a
    gid = sub_l * cfg.nblk_a + ablk_l
    ngrp = 2 * cfg.nblk_a
    gcounts = np.zeros((NCORES, ngrp), np.int64)
    per_core_lbl = []
    for k in range(NCORES):
        m = core_l == k
        lc, la, g, orig = l_c[m], l_a[m], gid[m], np.nonzero(m)[0]
        order = np.argsort(g, kind="stable")
        lc, la, g, orig = lc[order], la[order], g[order], orig[order]
        gcounts[k] = np.bincount(g, minlength=ngrp)
        per_core_lbl.append((lc, la, g, orig))
    grp_L = _ru(gcounts.max(axis=0), P)
    goffs = np.concatenate([[0], np.cumsum(grp_L)]).astype(np.int64)
    ld_pad = int(goffs[-1])

    # per-label customer offset within its sub (for Sel blocks), sorted by
    # customer within each group so the Uc side is a dense Sel-matmul
    dec_idx_a, out_pos = [], []
    lcrel_all = np.full((NCORES, ld_pad), -1.0e9, np.float64)
    for k in range(NCORES):
        lc, la, g, orig = per_core_lbl[k]
        lcl = lc % cfg.cpc
        lcr = lcl - (lcl // cfg.zc_sub) * cfg.zc_sub
        order = np.lexsort((lcr, g))
        lc, la, g, orig, lcr = (lc[order], la[order], g[order],
                                orig[order], lcr[order])
        gstart = np.concatenate([[0], np.cumsum(gcounts[k])])[:-1]
        pos = goffs[g] + (np.arange(len(lc)) - gstart[g])
        ia = np.zeros(ld_pad, np.int16)
        po = np.full(ld_pad, -1, np.int64)
        ia[pos] = (la - (la // cfg.srcb_a) * cfg.srcb_a).astype(np.int16)
        po[pos] = orig
        lcrel_all[k, pos] = lcr.astype(np.float64)
        dec_idx_a.append(_wrap_idx(ia))
        out_pos.append(po)

    # Sel emissions (union over cores) + per-core fp8 Sel streams
    subw = -(-cfg.zc_sub // P)             # cust windows per padded sub
    sel_ems = []                           # per group: list of (t, wlo, nw)
    for gi in range(ngrp):
        o, L = int(goffs[gi]), int(grp_L.reshape(-1)[gi])
        if L == 0:
            sel_ems.append([])
            continue
        sel_ems.append(_emissions(lcrel_all[:, o:o + L], L // P, P, subw))
    em_base, em_w = [], []
    for gi in range(ngrp):
        o = int(goffs[gi])
        for (t, a, nw) in sel_ems[gi]:
            for wi in range(nw):
                em_base.append(o + t * P)
                em_w.append(a + wi)
    em_base = np.array(em_base, np.int64)
    em_w = np.array(em_w, np.int64)
    sel_cols = _ru(max(len(em_base) * P, P), XCH)
    sels = []
    for k in range(NCORES):
        D = lcrel_all[k][em_base[:, None] + np.arange(P)[None, :]]
        D = np.where(D < -1.0, -1000.0, D)
        tgt = em_w[:, None] * P + np.arange(P)[None, :]
        blk = (D[:, :, None] == tgt[:, None, :])   # [ne, label, cust]
        ps = blk.transpose(2, 0, 1).reshape(P, -1)  # part=cust, col=label
        full = np.zeros((P, sel_cols), np.float16)
        full[:, :ps.shape[1]] = ps
        sels.append(full)

    dec = dict(grp_L=grp_L.reshape(2, cfg.nblk_a), goffs=goffs,
               ld_pad=ld_pad, idx_a=dec_idx_a, out_pos=out_pos,
               sel_ems=sel_ems, sel_cols=sel_cols, sels=sels, subw=subw)
    return pa1, pc1, pa2, pc2, dec


# ---------------------------------------------------------------------------
# kernel builder
# ---------------------------------------------------------------------------

F32 = mybir.dt.float32


def build_nc(cfg, pa1, pc1, pa2, pc2, dec):
    DT = cfg.dt
    nc = bacc.Bacc("TRN2", target_bir_lowering=False, debug=False,
                   num_devices=NCORES, num_swdge_queues=4)
    qctr = [0]

    def next_q():
        # 4-queue rotation lets the 8 GpSimd Q7 cores overlap descriptor
        # generation across gathers (~2x).  CoreSim's per-sem-lane queue
        # lock can't be satisfied under scheduler reordering; hardware
        # tolerates mixed queues per lane (sems are plain counters).
        if SIM_SINGLE_QUEUE:
            return 0
        qctr[0] = (qctr[0] + 1) % 4
        return qctr[0]

    ei = lambda n, s, d: nc.dram_tensor(n, s, d, kind="ExternalInput")
    xaT = ei("xaT", [P, cfg.apc], DT)
    xcT = ei("xcT", [P, cfg.cpc], DT)
    DT8 = cfg.dt8
    xA1 = ei("xA1", [P, pa1.etot], DT)
    xC1 = [ei(f"xC1{h}", [P, pc1[h].etot], DT) for h in range(2)]
    pA1 = ei("pA1", [P, pa1.p_cols], DT8)
    pC1 = [ei(f"pC1{h}", [P, pc1[h].p_cols], DT8) for h in range(2)]
    pA2 = ei("pA2", [P, pa2.p_cols], DT8)
    pC2 = [ei(f"pC2{h}", [P, pc2[h].p_cols], DT8) for h in range(2)]
    iA2 = ei("iA2", [P, pa2.etot // 16], mybir.dt.int16)
    iC2 = [ei(f"iC2{h}", [P, pc2[h].etot // 16], mybir.dt.int16)
           for h in range(2)]
    dec_idx_a = ei("dec_idx_a", [P, dec["ld_pad"] // 16], mybir.dt.int16)
    selD = ei("selD", [P, dec["sel_cols"]], cfg.dt8)
    sclA = ei("sclA", [1, cfg.apc], DT)
    sclC = ei("sclC", [1, cfg.cpc], DT)

    wnames = ["W_msg1_ca", "W_self1_a", "W_msg1_ac", "W_self1_c",
              "W_msg2_ca", "W_self2_a", "W_msg2_ac", "W_self2_c",
              "Wd1c", "Wd1a"]
    wts = {n: ei(n, [P, P], DT) for n in wnames}
    w2rep = ei("w2rep", [P, DGCH], DT)
    bnames = ["b1_a", "b1_c", "b2_a", "b2_c",
              "bn_gamma_c", "bn_beta_c", "bn_gamma_a", "bn_beta_a",
              "b_dec1", "b_dec2c"]
    bis = {n: ei(n, [P, 1], F32) for n in bnames}

    ldT = dec["ld_pad"] // P
    y_out = nc.dram_tensor("y", [P, ldT], F32, kind="ExternalOutput")
    rg = [list(range(NCORES))]

    with tile.TileContext(nc) as tc:
        with (
            tc.tile_pool(name="dramp", bufs=1, space="DRAM") as dramp,
            tc.tile_pool(name="const", bufs=1) as cs,
        ):
            ha_own = dramp.tile([cfg.apc, P], DT)
            ha_full = dramp.tile([cfg.n_a, P], DT, addr_space="Shared")
            hc_own = [dramp.tile([cfg.chalf, P], DT, name=f"hc_own{h}")
                      for h in range(2)]
            hc_full = [dramp.tile([cfg.n_c // 2, P], DT,
                                  addr_space="Shared", name=f"hc_full{h}")
                       for h in range(2)]
            ua_own = dramp.tile([cfg.apc, P], DT)
            ua_full = dramp.tile([cfg.n_a, P], DT, addr_space="Shared")
            uc_pad = dramp.tile([2 * dec["subw"] * P, P], DT)
            haT_d = dramp.tile([P, cfg.apc], DT)
            hcT_d = dramp.tile([P, cfg.cpc], DT)
            zaT_d = dramp.tile([P, cfg.apc], DT)
            zcT_d = dramp.tile([P, cfg.cpc], DT)
            stats_in_a = dramp.tile([P, 2], F32)
            stats_out_a = dramp.tile([P, 2], F32, addr_space="Shared")
            stats_in_c = dramp.tile([P, 2], F32)
            stats_out_c = dramp.tile([P, 2], F32, addr_space="Shared")

            ident = cs.tile([P, P], DT)
            make_identity(nc, ident[:])
            w_sb = {n: cs.tile([P, P], DT, name=f"w_{n}") for n in wnames}
            for n in wnames:
                nc.sync.dma_start(out=w_sb[n][:], in_=wts[n][:])
            w2r_sb = cs.tile([P, DGCH], DT)
            nc.sync.dma_start(out=w2r_sb[:], in_=w2rep[:])
            b_sb = {n: cs.tile([P, 1], F32, name=f"b_{n}") for n in bnames}
            for n in bnames:
                nc.sync.dma_start(out=b_sb[n][:], in_=bis[n][:])
            stats_sb = cs.tile([P, 4], F32)
            nc.vector.memset(stats_sb[:], 0.0)
            ones1 = cs.tile([1, P], DT)
            nc.vector.memset(ones1[:], 1.0)

            # ------------- per-dst 1/cnt replicated across partitions ---
            def build_sclrep(scl_dram, off, nloc, sbp, psp):
                srow = sbp.tile([1, nloc], DT, tag="srow", name="srow",
                                bufs=1)
                nc.sync.dma_start(out=srow[:], in_=scl_dram[:, off:off + nloc])
                srep = sbp.tile([P, _ru(nloc, BANK)], DT, tag="srep",
                                name="srep", bufs=1)
                for c0 in range(0, nloc, BANK):
                    bw = min(BANK, nloc - c0)
                    sp = psp.tile([P, BANK], F32, tag="wps", name="sclps",
                                  bufs=2)
                    nc.tensor.matmul(sp[:, :bw], lhsT=ones1[:],
                                     rhs=srow[:, c0:c0 + bw], start=True,
                                     stop=True, skip_group_check=True)
                    nc.scalar.copy(srep[:, c0:c0 + bw], sp[:, :bw])
                return srep

            # ------------- shared W-stage (per superblock) -------------
            def w_stage_sb(meanT_sb, c0, cw, selfT_dram, self_off, wmsg,
                           wself, bias_col, relu, outT_dram, outT_off,
                           rows_dram, rows_off, stats_cols, sbp, psp):
                """One 512-col (max) chunk loop over [c0, c0+cw)."""
                for b0 in range(0, cw, BANK):
                    bw = min(BANK, cw - b0)
                    sT = sbp.tile([P, BANK], DT, tag="wself", name="wselfT",
                                  bufs=3)
                    nc.sync.dma_start(
                        out=sT[:, :bw],
                        in_=selfT_dram[:, self_off + c0 + b0:
                                       self_off + c0 + b0 + bw])
                    psum = psp.tile([P, BANK], F32, tag="wps", name="wps",
                                    bufs=2)
                    nc.tensor.matmul(psum[:, :bw], lhsT=wmsg,
                                     rhs=meanT_sb[:, c0 + b0:c0 + b0 + bw],
                                     start=True, stop=False,
                                     skip_group_check=True)
                    nc.tensor.matmul(psum[:, :bw], lhsT=wself,
                                     rhs=sT[:, :bw], start=False, stop=True,
                                     skip_group_check=True)
                    oT = sbp.tile([P, BANK], DT, tag="woT", name="woT",
                                  bufs=3)
                    nc.scalar.activation(
                        oT[:, :bw], psum[:, :bw],
                        mybir.ActivationFunctionType.Relu if relu
                        else mybir.ActivationFunctionType.Identity,
                        bias=bias_col[:], scale=1.0)
                    nc.sync.dma_start(
                        out=outT_dram[:, outT_off + c0 + b0:
                                      outT_off + c0 + b0 + bw],
                        in_=oT[:, :bw])
                    if stats_cols is not None:
                        si, sj = stats_cols
                        part = sbp.tile([P, 1], F32, tag="wst1", name="wst1",
                                        bufs=2)
                        nc.vector.reduce_sum(part[:], oT[:, :bw],
                                             mybir.AxisListType.X)
                        nc.vector.tensor_add(stats_sb[:, si:si + 1],
                                             stats_sb[:, si:si + 1],
                                             part[:])
                        trash = sbp.tile([P, BANK], F32, tag="wtrash",
                                         name="wtrash", bufs=2)
                        part2 = sbp.tile([P, 1], F32, tag="wst2",
                                         name="wst2", bufs=2)
                        nc.scalar.activation(
                            trash[:, :bw], oT[:, :bw],
                            mybir.ActivationFunctionType.Square,
                            accum_out=part2[:])
                        nc.vector.tensor_add(stats_sb[:, sj:sj + 1],
                                             stats_sb[:, sj:sj + 1],
                                             part2[:])
                    if rows_dram is not None:
                        _emit_rows(oT, bw, rows_dram, rows_off + c0 + b0,
                                   sbp, psp)

            def _emit_rows(srcT_sb, cw, rows_dram, row_base, sbp, psp):
                """Transpose [128, cw<=512] -> cw rows in DRAM, batched."""
                rows = sbp.tile([P, BANK], DT, tag="rows", name="rows",
                                bufs=3)
                nb = -(-cw // P)
                for i in range(nb):
                    b0 = i * P
                    bw = min(P, cw - b0)
                    tp = psp.tile([P, P], DT, tag="tps", name="tps", bufs=2)
                    nc.tensor.transpose(tp[:bw, :], srcT_sb[:, b0:b0 + bw],
                                        ident[:])
                    nc.scalar.copy(rows[:bw, i * P:(i + 1) * P], tp[:bw, :])
                out_ap = rows_dram[row_base:row_base + cw, :]
                if cw == nb * P:
                    out_ap = out_ap.rearrange("(c p) f -> p c f", p=P)
                    nc.sync.dma_start(out=out_ap, in_=rows[:, :cw]
                                      .rearrange("p (c f) -> p c f", f=P))
                else:
                    for i in range(nb):
                        b0 = i * P
                        bw = min(P, cw - b0)
                        nc.sync.dma_start(
                            out=rows_dram[row_base + b0:row_base + b0 + bw,
                                          :],
                            in_=rows[:bw, i * P:i * P + P])

            # ------------- L1 aggregation (streamed, no gathers) --------
            def agg_l1(ps, x_d, p_d, meanT_sb, srep, sbp, psp, wargs):
                W = ps.W
                xoff = 0
                poff = 0
                for s in range(ps.nsb):
                    nodes = ps.sb_nodes[s]
                    L = int(ps.sb_len[s])
                    T = L // P
                    psum = psp.tile([P, cfg.sbn], F32, tag="aggps",
                                    name="aggps", bufs=2)
                    ems = ps.emits[s]
                    # bank bookkeeping
                    firsts, lasts, banks = {}, {}, set()
                    seq = []
                    for (t, a, nw) in ems:
                        for wi in range(nw):
                            w = a + wi
                            b = (w * W) // BANK
                            seq.append((t, w, b))
                            banks.add(b)
                    for i, (t, w, b) in enumerate(seq):
                        if b not in firsts:
                            firsts[b] = i
                        lasts[b] = i
                    # X stream chunks covering this sb
                    x_tiles = {}
                    for c0 in range(0, L, XCH):
                        cl = min(XCH, L - c0)
                        xt = sbp.tile([P, XCH], DT, tag="xs", name="xs",
                                      bufs=3)
                        nc.sync.dma_start(
                            out=xt[:, :cl],
                            in_=x_d[:, xoff + c0:xoff + c0 + cl])
                        x_tiles[c0 // XCH] = xt
                    # P stream chunks
                    pcols = sum(e[2] for e in ems) * W
                    p_tiles = {}
                    for c0 in range(0, pcols, XCH):
                        cl = min(XCH, pcols - c0)
                        pt = sbp.tile([P, XCH], cfg.dt8, tag="pstr",
                                      name="pstr", bufs=3)
                        nc.sync.dma_start(
                            out=pt[:, :cl],
                            in_=p_d[:, poff + c0:poff + c0 + cl])
                        p_tiles[c0 // XCH] = pt
                    pcur = 0
                    for i, (t, w, b) in enumerate(seq):
                        xt = x_tiles[(t * P) // XCH]
                        xsl = xt[:, (t * P) % XCH:(t * P) % XCH + P]
                        pt = p_tiles[pcur // XCH]
                        psl = pt[:, pcur % XCH:pcur % XCH + W]
                        nc.tensor.matmul(
                            psum[:, w * W:(w + 1) * W], lhsT=xsl, rhs=psl,
                            start=(firsts[b] == i), stop=(lasts[b] == i),
                            skip_group_check=True)
                        pcur += W
                    _sb_finish(psum, banks, nodes, s, meanT_sb, srep)
                    xoff += L
                    poff += pcols
                    w_stage_sb(meanT_sb, s * cfg.sbn, nodes, *wargs,
                               sbp=sbp, psp=psp)

            def _sb_finish(psum, banks, nodes, s, meanT_sb, srep):
                """meanT = psum * sclrep (DVE) + memset uncovered banks."""
                nbank = -(-nodes // BANK)
                for b in range(nbank):
                    a = b * BANK
                    e = min((b + 1) * BANK, nodes)
                    dst = meanT_sb[:, s * cfg.sbn + a:s * cfg.sbn + e]
                    if b in banks:
                        nc.vector.tensor_tensor(
                            out=dst, in0=psum[:, a:e],
                            in1=srep[:, s * cfg.sbn + a:s * cfg.sbn + e],
                            op=mybir.AluOpType.mult)
                    else:
                        nc.vector.memset(dst, 0.0)

            # ------------- L2 aggregation (gather + shipped P) ----------
            def agg_l2(ps, tables, idx_d, p_d, meanT_sb, srep, sbp, psp,
                       wargs):
                """tables: list of (j_lo, j_hi, dram_tile, row_off, rows)."""
                W = ps.W
                poff = 0
                for s in range(ps.nsb):
                    nodes = ps.sb_nodes[s]
                    psum = psp.tile([P, cfg.sbn], F32, tag="aggps",
                                    name="aggps", bufs=2)
                    ems = ps.emits[s]
                    firsts, lasts, banks = {}, {}, set()
                    seq = []
                    for (j, t, a, nw) in ems:
                        for wi in range(nw):
                            w = a + wi
                            b = (w * W) // BANK
                            seq.append((j, t, w, b))
                            banks.add(b)
                    for i, (j, t, w, b) in enumerate(seq):
                        if b not in firsts:
                            firsts[b] = i
                        lasts[b] = i
                    # per-run gathers
                    x_runs = {}
                    rmax = int(ps.run_L.max())
                    for j in range(ps.nblk):
                        L = int(ps.run_L[s, j])
                        if L == 0:
                            continue
                        o = int(ps.offs[s * ps.nblk + j])
                        idx_sb = sbp.tile([P, rmax // 16], mybir.dt.int16,
                                          tag="gidx", name="gidx", bufs=4)
                        nc.sync.dma_start(
                            out=idx_sb[:, :L // 16],
                            in_=idx_d[:, o // 16:(o + L) // 16])
                        xg = sbp.tile([P, rmax // P, P], DT, tag="gx",
                                      name="gx", bufs=6)
                        done = False
                        for (jlo, jhi, tbl, roff, trows) in tables:
                            if jlo <= j <= jhi:
                                rel = j * ps.srcb - roff
                                blk_rows = min(ps.srcb, trows - rel)
                                for c0 in range(0, L, 1024):
                                    cl = min(1024, L - c0)
                                    nc.gpsimd.dma_gather(
                                        xg[:, c0 // P:(c0 + cl) // P, :],
                                        tbl[rel:rel + blk_rows, :],
                                        idx_sb[:, c0 // 16:(c0 + cl) // 16],
                                        cl, cl, P, queue_num=next_q())
                                done = True
                                break
                        assert done
                        x_runs[j] = xg
                    # P stream chunks
                    pcols = sum(e[3] for e in ems) * W
                    p_tiles = {}
                    for c0 in range(0, pcols, XCH):
                        cl = min(XCH, pcols - c0)
                        pt = sbp.tile([P, XCH], cfg.dt8, tag="pstr",
                                      name="pstr", bufs=3)
                        nc.sync.dma_start(
                            out=pt[:, :cl],
                            in_=p_d[:, poff + c0:poff + c0 + cl])
                        p_tiles[c0 // XCH] = pt
                    pcur = 0
                    for i, (j, t, w, b) in enumerate(seq):
                        xg = x_runs[j]
                        pt = p_tiles[pcur // XCH]
                        psl = pt[:, pcur % XCH:pcur % XCH + W]
                        nc.tensor.matmul(
                            psum[:, w * W:(w + 1) * W], lhsT=xg[:, t, :],
                            rhs=psl, start=(firsts[b] == i),
                            stop=(lasts[b] == i), skip_group_check=True)
                        pcur += W
                    _sb_finish(psum, banks, nodes, s, meanT_sb, srep)
                    poff += pcols
                    w_stage_sb(meanT_sb, s * cfg.sbn, nodes, *wargs,
                               sbp=sbp, psp=psp)

            # ================= layer 1 =================
            with (
                tc.tile_pool(name="l1a", bufs=1) as sbp,
                tc.tile_pool(name="l1ap", bufs=1, space="PSUM") as psp,
                tc.tile_pool(name="l1am", bufs=1) as mp,
            ):
                meanT = mp.tile([P, pa1.nsb * cfg.sbn], DT, name="meanTA")
                srep = build_sclrep(sclA, 0, cfg.apc, sbp, psp)
                agg_l1(pa1, xA1, pA1, meanT, srep, sbp, psp,
                       (xaT, 0, w_sb["W_msg1_ca"][:], w_sb["W_self1_a"][:],
                        b_sb["b1_a"], True, haT_d, 0, ha_own, 0, None))
            nc.gpsimd.collective_compute(
                "AllGather", mybir.AluOpType.bypass, replica_groups=rg,
                ins=[ha_own[:]], outs=[ha_full[:]])

            for h in range(2):
                with (
                    tc.tile_pool(name=f"l1c{h}", bufs=1) as sbp,
                    tc.tile_pool(name=f"l1cp{h}", bufs=1,
                                 space="PSUM") as psp,
                    tc.tile_pool(name=f"l1cm{h}", bufs=1) as mp,
                ):
                    meanT = mp.tile([P, pc1[h].nsb * cfg.sbn], DT,
                                    name="meanTC")
                    srep = build_sclrep(sclC, h * cfg.chalf, cfg.chalf,
                                        sbp, psp)
                    agg_l1(pc1[h], xC1[h], pC1[h], meanT, srep, sbp, psp,
                           (xcT, h * cfg.chalf, w_sb["W_msg1_ac"][:],
                            w_sb["W_self1_c"][:], b_sb["b1_c"], True,
                            hcT_d, h * cfg.chalf, hc_own[h], 0, None))
                nc.gpsimd.collective_compute(
                    "AllGather", mybir.AluOpType.bypass, replica_groups=rg,
                    ins=[hc_own[h][:]], outs=[hc_full[h][:]])

            # ================= layer 2 =================
            half_rows = cfg.n_c // 2
            nj_half = half_rows // cfg.srcb_c
            a2_tables = [(0, nj_half - 1, hc_full[0], 0, half_rows),
                         (nj_half, cfg.nblk_c - 1, hc_full[1], half_rows,
                          half_rows)]
            with (
                tc.tile_pool(name="l2a", bufs=1) as sbp,
                tc.tile_pool(name="l2ap", bufs=1, space="PSUM") as psp,
                tc.tile_pool(name="l2am", bufs=1) as mp,
            ):
                meanT = mp.tile([P, pa2.nsb * cfg.sbn], DT, name="meanTA2")
                srep = build_sclrep(sclA, 0, cfg.apc, sbp, psp)
                agg_l2(pa2, a2_tables, iA2, pA2, meanT, srep, sbp, psp,
                       (haT_d, 0, w_sb["W_msg2_ca"][:], w_sb["W_self2_a"][:],
                        b_sb["b2_a"], False, zaT_d, 0, None, 0, (0, 1)))
            nc.sync.dma_start(out=stats_in_a[:], in_=stats_sb[:, 0:2])
            nc.gpsimd.collective_compute(
                "AllReduce", mybir.AluOpType.add, replica_groups=rg,
                ins=[stats_in_a[:]], outs=[stats_out_a[:]])
            c2_tables = [(0, cfg.nblk_a - 1, ha_full, 0, cfg.n_a)]
            for h in range(2):
                with (
                    tc.tile_pool(name=f"l2c{h}", bufs=1) as sbp,
                    tc.tile_pool(name=f"l2cp{h}", bufs=1,
                                 space="PSUM") as psp,
                    tc.tile_pool(name=f"l2cm{h}", bufs=1) as mp,
                ):
                    meanT = mp.tile([P, pc2[h].nsb * cfg.sbn], DT,
                                    name="meanTC2")
                    srep = build_sclrep(sclC, h * cfg.chalf, cfg.chalf,
                                        sbp, psp)
                    agg_l2(pc2[h], c2_tables, iC2[h], pC2[h], meanT, srep,
                           sbp, psp,
                           (hcT_d, h * cfg.chalf, w_sb["W_msg2_ac"][:],
                            w_sb["W_self2_c"][:], b_sb["b2_c"], False,
                            zcT_d, h * cfg.chalf, None, 0, (2, 3)))

            # ================= BN + U tables =================
            with (
                tc.tile_pool(name="bn", bufs=1) as sbp,
                tc.tile_pool(name="bnp", bufs=1, space="PSUM") as psp,
            ):
                nc.sync.dma_start(out=stats_in_c[:], in_=stats_sb[:, 2:4])
                nc.gpsimd.collective_compute(
                    "AllReduce", mybir.AluOpType.add, replica_groups=rg,
                    ins=[stats_in_c[:]], outs=[stats_out_c[:]])
                st = sbp.tile([P, 4], F32)
                nc.sync.dma_start(out=st[:, 0:2], in_=stats_out_a[:])
                nc.sync.dma_start(out=st[:, 2:4], in_=stats_out_c[:])

                def bn_coeff(si, sj, n, gamma, beta, tagp):
                    mu = sbp.tile([P, 1], F32, name=f"mu{tagp}")
                    nc.vector.tensor_scalar_mul(mu[:], st[:, si:si + 1],
                                                1.0 / n)
                    msq = sbp.tile([P, 1], F32, name=f"msq{tagp}")
                    nc.vector.tensor_scalar_mul(msq[:], st[:, sj:sj + 1],
                                                1.0 / n)
                    mu2 = sbp.tile([P, 1], F32, name=f"mu2{tagp}")
                    nc.vector.tensor_mul(mu2[:], mu[:], mu[:])
                    var = sbp.tile([P, 1], F32, name=f"var{tagp}")
                    nc.vector.tensor_sub(var[:], msq[:], mu2[:])
                    nc.vector.tensor_scalar_add(var[:], var[:], BN_EPS)
                    sd = sbp.tile([P, 1], F32, name=f"sd{tagp}")
                    nc.scalar.activation(sd[:], var[:],
                                         mybir.ActivationFunctionType.Sqrt)
                    rstd = sbp.tile([P, 1], F32, name=f"rstd{tagp}")
                    nc.vector.reciprocal(rstd[:], sd[:])
                    scl = sbp.tile([P, 1], F32, name=f"scl{tagp}")
                    nc.vector.tensor_mul(scl[:], b_sb[gamma][:], rstd[:])
                    mg = sbp.tile([P, 1], F32, name=f"mg{tagp}")
                    nc.vector.tensor_mul(mg[:], mu[:], scl[:])
                    bia = sbp.tile([P, 1], F32, name=f"bia{tagp}")
                    nc.vector.tensor_sub(bia[:], b_sb[beta][:], mg[:])
                    return scl, bia

                scl_a_c, bia_a_c = bn_coeff(0, 1, cfg.n_a, "bn_gamma_a",
                                            "bn_beta_a", "a")

                def bn_u(nloc, zT_dram, scl, bia, w1half, ubias, rows_dram):
                    bn_u_sub(nloc, zT_dram, 0, scl, bia, w1half, ubias,
                             rows_dram, 0)

                def bn_u_sub(nloc, zT_dram, z_off, scl, bia, w1half, ubias,
                             rows_dram, r_off):
                    for c0 in range(0, nloc, BANK):
                        cw = min(BANK, nloc - c0)
                        zT = sbp.tile([P, BANK], DT, tag="bnz", name="bnz",
                                      bufs=3)
                        nc.sync.dma_start(
                            out=zT[:, :cw],
                            in_=zT_dram[:, z_off + c0:z_off + c0 + cw])
                        bnT = sbp.tile([P, BANK], DT, tag="bnt", name="bnt",
                                       bufs=3)
                        nc.scalar.activation(
                            bnT[:, :cw], zT[:, :cw],
                            mybir.ActivationFunctionType.Identity,
                            bias=bia[:], scale=scl[:])
                        ups = psp.tile([P, BANK], F32, tag="ups",
                                       name="ups", bufs=2)
                        nc.tensor.matmul(ups[:, :cw], lhsT=w1half,
                                         rhs=bnT[:, :cw], start=True,
                                         stop=True, skip_group_check=True)
                        uT = sbp.tile([P, BANK], DT, tag="uT", name="uT",
                                      bufs=3)
                        nc.scalar.activation(
                            uT[:, :cw], ups[:, :cw],
                            mybir.ActivationFunctionType.Identity,
                            bias=ubias[:] if ubias is not None else 0.0,
                            scale=1.0)
                        _emit_rows(uT, cw, rows_dram, r_off + c0, sbp, psp)

                bn_u(cfg.apc, zaT_d, scl_a_c, bia_a_c, w_sb["Wd1a"][:],
                     None, ua_own)
                nc.gpsimd.collective_compute(
                    "AllGather", mybir.AluOpType.bypass, replica_groups=rg,
                    ins=[ua_own[:]], outs=[ua_full[:]])
                scl_c_c, bia_c_c = bn_coeff(2, 3, cfg.n_c, "bn_gamma_c",
                                            "bn_beta_c", "c")
                zpad = sbp.tile([P, P], DT, name="zpad")
                nc.vector.memset(zpad[:], 0.0)
                for sub in range(2):
                    bn_u_sub(cfg.zc_sub, zcT_d, sub * cfg.zc_sub, scl_c_c,
                             bia_c_c, w_sb["Wd1c"][:], b_sb["b_dec1"],
                             uc_pad, sub * dec["subw"] * P)
                    npad = dec["subw"] * P - cfg.zc_sub
                    if npad > 0:
                        nc.sync.dma_start(
                            out=uc_pad[sub * dec["subw"] * P + cfg.zc_sub:
                                       (sub + 1) * dec["subw"] * P, :],
                            in_=zpad[:npad, :])

            # ================= decoder =================
            grp_L = dec["grp_L"]
            goffs = dec["goffs"]
            subw = dec["subw"]
            with (
                tc.tile_pool(name="dec", bufs=1) as sbp,
                tc.tile_pool(name="decp", bufs=1, space="PSUM") as psp,
            ):
                ysb = sbp.tile([P, ldT], F32, name="ysb")
                scur = 0          # Sel stream cursor (cols)
                s_tiles = {}

                def sel_chunk(c0):
                    ci = c0 // XCH
                    if ci not in s_tiles:
                        st = sbp.tile([P, XCH], cfg.dt8, tag="sel",
                                      name="sel", bufs=3)
                        nc.sync.dma_start(
                            out=st[:], in_=selD[:, ci * XCH:(ci + 1) * XCH])
                        s_tiles[ci] = st
                    return s_tiles[ci]

                for sub in range(2):
                    ucres = sbp.tile([P, subw * P], DT, tag="ucres",
                                     name="ucres", bufs=1)
                    nc.sync.dma_start(
                        out=ucres[:].rearrange("p (w f) -> p w f", f=P),
                        in_=uc_pad[sub * subw * P:(sub + 1) * subw * P, :]
                        .rearrange("(w p) f -> p w f", p=P))
                    for ab in range(cfg.nblk_a):
                        gi = sub * cfg.nblk_a + ab
                        L = int(grp_L[sub, ab])
                        o = int(goffs[gi])
                        ua_rows = min(cfg.srcb_a,
                                      cfg.n_a - ab * cfg.srcb_a)
                        ems = dec["sel_ems"][gi]
                        by_tile = {}
                        for (t, a, nw) in ems:
                            by_tile[t] = (a, nw)
                        for c0 in range(0, L, DGCH):
                            cl = min(DGCH, L - c0)
                            oc = o + c0
                            ctn = cl // P
                            ixa = sbp.tile([P, DGCH // 16], mybir.dt.int16,
                                           tag="dixa", name="dixa", bufs=4)
                            nc.sync.dma_start(
                                out=ixa[:, :cl // 16],
                                in_=dec_idx_a[:, oc // 16:(oc + cl) // 16])
                            uag = sbp.tile([P, DGCH // P, P], DT, tag="dua",
                                           name="dua", bufs=4)
                            for g0 in range(0, cl, 1024):
                                gl = min(1024, cl - g0)
                                nc.gpsimd.dma_gather(
                                    uag[:, g0 // P:(g0 + gl) // P, :],
                                    ua_full[ab * cfg.srcb_a:
                                            ab * cfg.srcb_a + ua_rows, :],
                                    ixa[:, g0 // 16:(g0 + gl) // 16],
                                    gl, gl, P, queue_num=next_q())
                            ssum = sbp.tile([P, DGCH], DT, tag="dsum",
                                            name="dsum", bufs=3)
                            # Uc side: Sel-matmul quads (4 tiles per bank)
                            for q0 in range(0, ctn, 4):
                                qn = min(4, ctn - q0)
                                dps = psp.tile([P, 4 * P], F32, tag="dps",
                                               name="dps", bufs=2)
                                mm = []
                                for ti in range(qn):
                                    t = (c0 // P) + q0 + ti
                                    if t not in by_tile:
                                        continue
                                    a, nw = by_tile[t]
                                    for wi in range(nw):
                                        mm.append((ti, a + wi))
                                for i, (ti, w) in enumerate(mm):
                                    st = sel_chunk(scur)
                                    ssl = st[:, scur % XCH:scur % XCH + P]
                                    nc.tensor.matmul(
                                        dps[:, ti * P:(ti + 1) * P],
                                        lhsT=ssl,
                                        rhs=ucres[:, w * P:(w + 1) * P],
                                        start=(i == 0), stop=(i == len(mm) - 1),
                                        skip_group_check=True)
                                    scur += P
                                if not mm:
                                    nc.vector.memset(dps[:], 0.0)
                                nc.vector.tensor_tensor(
                                    out=ssum[:, (q0 * P):(q0 + qn) * P]
                                    .rearrange("p (t w) -> p t w", w=P),
                                    in0=dps[:, :qn * P]
                                    .rearrange("p (t w) -> p t w", w=P),
                                    in1=uag[:, q0:q0 + qn, :],
                                    op=mybir.AluOpType.add)
                            rel = sbp.tile([P, DGCH], DT, tag="drel",
                                           name="drel", bufs=3)
                            nc.scalar.activation(
                                rel[:, :cl], ssum[:, :cl],
                                mybir.ActivationFunctionType.Relu)
                            nc.vector.tensor_mul(
                                rel[:, :cl], rel[:, :cl], w2r_sb[:, :cl])
                            nc.vector.reduce_sum(
                                ysb[:, oc // P:oc // P + ctn],
                                rel[:, :cl].rearrange(
                                    "p (t w) -> p t w", w=P),
                                mybir.AxisListType.X)
                nc.vector.tensor_scalar(
                    out=ysb[:], in0=ysb[:], scalar1=b_sb["b_dec2c"][:],
                    scalar2=None, op0=mybir.AluOpType.add)
                nc.sync.dma_start(out=y_out[:], in_=ysb[:])

    nc.compile()
    return nc


# ---------------------------------------------------------------------------
# entry point
# ---------------------------------------------------------------------------

def _pack_x(xrows, src_order):
    """[etot] indices into xrows [n, 128] -> [128, etot] tile-major pack."""
    g = xrows[src_order]                       # [etot, 128]
    T = g.shape[0] // P
    return np.ascontiguousarray(
        g.reshape(T, P, P).transpose(1, 0, 2).reshape(P, T * P))


def make_in_maps(cfg, inputs, pa1, pc1, pa2, pc2, dec):
    npdt = cfg.npdt
    f = lambda a: np.ascontiguousarray(np.asarray(a), dtype=np.float32)
    xc16 = f(inputs["x_customer"]).astype(npdt)
    xa16 = f(inputs["x_article"]).astype(npdt)
    wd1 = f(inputs["W_dec1"])
    w2 = f(inputs["W_dec2"]).reshape(-1)
    base = dict(
        W_msg1_ca=f(inputs["W_msg1_ca"]).astype(npdt),
        W_self1_a=f(inputs["W_self1_a"]).astype(npdt),
        W_msg1_ac=f(inputs["W_msg1_ac"]).astype(npdt),
        W_self1_c=f(inputs["W_self1_c"]).astype(npdt),
        W_msg2_ca=f(inputs["W_msg2_ca"]).astype(npdt),
        W_self2_a=f(inputs["W_self2_a"]).astype(npdt),
        W_msg2_ac=f(inputs["W_msg2_ac"]).astype(npdt),
        W_self2_c=f(inputs["W_self2_c"]).astype(npdt),
        Wd1c=wd1[:P].astype(npdt), Wd1a=wd1[P:].astype(npdt),
        w2rep=np.tile(w2.astype(npdt).reshape(1, P), (P, DGCH // P)),
        b1_a=f(inputs["b1_a"]).reshape(P, 1),
        b1_c=f(inputs["b1_c"]).reshape(P, 1),
        b2_a=f(inputs["b2_a"]).reshape(P, 1),
        b2_c=f(inputs["b2_c"]).reshape(P, 1),
        bn_gamma_c=f(inputs["bn_gamma_c"]).reshape(P, 1),
        bn_beta_c=f(inputs["bn_beta_c"]).reshape(P, 1),
        bn_gamma_a=f(inputs["bn_gamma_a"]).reshape(P, 1),
        bn_beta_a=f(inputs["bn_beta_a"]).reshape(P, 1),
        b_dec1=f(inputs["b_dec1"]).reshape(P, 1),
        b_dec2c=np.full((P, 1),
                        float(np.asarray(inputs["b_dec2"]).item()),
                        np.float32),
    )
    e_dst = np.asarray(inputs["edge_dst_article"]).astype(np.int64)
    e_srcc = np.asarray(inputs["edge_src_customer"]).astype(np.int64)
    cnt_a = np.bincount(e_dst, minlength=cfg.n_a)
    cnt_c = np.bincount(e_srcc, minlength=cfg.n_c)
    scl_a16 = (1.0 / np.maximum(cnt_a, 1.0)).astype(npdt)
    scl_c16 = (1.0 / np.maximum(cnt_c, 1.0)).astype(npdt)
    in_maps = []
    for k in range(NCORES):
        m = dict(base)
        m["xaT"] = np.ascontiguousarray(
            xa16[k * cfg.apc:(k + 1) * cfg.apc].T)
        m["xcT"] = np.ascontiguousarray(
            xc16[k * cfg.cpc:(k + 1) * cfg.cpc].T)
        m["sclA"] = scl_a16[k * cfg.apc:(k + 1) * cfg.apc].reshape(1, -1)
        m["sclC"] = scl_c16[k * cfg.cpc:(k + 1) * cfg.cpc].reshape(1, -1)
        np8 = cfg.np8
        m["xA1"] = _pack_x(xc16, pa1.src_order[k])
        m["pA1"] = pa1.pstreams[k].astype(np8)
        for h in range(2):
            m[f"xC1{h}"] = _pack_x(xa16, pc1[h].src_order[k])
            m[f"pC1{h}"] = pc1[h].pstreams[k].astype(np8)
            m[f"pC2{h}"] = pc2[h].pstreams[k].astype(np8)
            m[f"iC2{h}"] = pc2[h].idx[k]
        m["pA2"] = pa2.pstreams[k].astype(np8)
        m["iA2"] = pa2.idx[k]
        m["dec_idx_a"] = dec["idx_a"][k]
        m["selD"] = dec["sels"][k].astype(np8)
        in_maps.append(m)
    return in_maps


def run(cfg, inputs, trace=False):
    pa1, pc1, pa2, pc2, dec = prep_all(cfg, inputs)
    in_maps = make_in_maps(cfg, inputs, pa1, pc1, pa2, pc2, dec)
    nc = build_nc(cfg, pa1, pc1, pa2, pc2, dec)
    res = run_bass_kernel_spmd(nc, in_maps, core_ids=list(range(NCORES)),
                               trace=trace)
    y = np.empty(cfg.e_lbl, np.float32)
    for k in range(NCORES):
        yl = res.results[k]["y"].T.reshape(-1)
        po = dec["out_pos"][k]
        vm = po >= 0
        y[po[vm]] = yl[vm]
    return y, res


def kernel(**inputs):
    cfg = Cfg()
    y, _ = run(cfg, inputs, trace=False)
    return y


# revision 29
# speedup vs baseline: 1.1284x; 1.1284x over previous
"""Hetero GNN encoder/decoder (SAGE x2 + BN + edge MLP decoder) on 8 trn2 cores.

v2 strategy (edge sharding by destination, node-range sharding):
  - Articles: core k owns rows [k*APC, (k+1)*APC); customers likewise (CPC,
    split in two halves per core).
  - All one-hot scatter matrices ("P strips") are STATIC edge structure and
    are precomputed on the host and shipped as inputs (fp16, 1/cnt scale
    folded in).  No DVE one-hot builds on device at all.
  - Layer 1 needs x_customer/x_article rows in edge order: these are also a
    static permutation of the inputs, so the host ships packed X streams and
    layer 1 runs with ZERO runtime gathers: stream X+P from DRAM, matmul
    into PSUM superblocks (feat on partitions, dst nodes on columns).
  - Layer 2 gathers h rows at runtime via one dma_gather per (sb, src-block)
    run (big calls amortize the ~1us SWDGE fixed overhead), then the same
    shipped-P matmul scatter.
  - hc is AllGathered in two halves (half-major relabeled row space) so the
    second AG overlaps the first half of layer-2 work.
  - Decoder uses precomputed U_c/U_a tables; per label
    y = w2 . relu(U_c[lc]+U_a[la]) + b2 with 4096-label gather chunks; relu
    on the scalar engine.

All structure (loop bounds, emissions) is compile-time and identical across
cores; per-core variation lives in the data (padded to uniform sizes).
"""
import sys

sys.path.insert(0, "/opt/trn_rl_repo")

import ml_dtypes
import numpy as np

import concourse.bacc as bacc
import concourse.bass as bass
import concourse.mybir as mybir
import concourse.tile as tile
from concourse.bass_utils import run_bass_kernel_spmd
from concourse.masks import make_identity

P = 128
NCORES = 8
BN_EPS = 1e-5
SIM_SINGLE_QUEUE = False   # set True when validating under CoreSim
DGCH = 4096           # decoder gather chunk
XCH = 8192            # L1 X/P stream load chunk (columns)
BANK = 512            # psum fp32 cols per bank


class Cfg:
    def __init__(self, n_c=300000, n_a=100000, e_lbl=1000000,
                 sbn=1024, w1a=16, w1c=16, w2a=32, w2c=32,
                 srcb_c=30000, srcb_a=25000):
        self.n_c, self.n_a, self.e_lbl = n_c, n_a, e_lbl
        self.cpc, self.apc = n_c // NCORES, n_a // NCORES
        assert self.cpc * NCORES == n_c and self.apc * NCORES == n_a
        self.chalf = self.cpc // 2
        self.sbn = sbn
        assert sbn % BANK == 0
        self.w1a, self.w1c, self.w2a, self.w2c = w1a, w1c, w2a, w2c
        self.srcb_c, self.srcb_a = srcb_c, srcb_a
        self.nblk_c = -(-n_c // srcb_c)
        self.nblk_a = -(-n_a // srcb_a)
        assert srcb_c < 32768 and srcb_a < 32768
        assert (n_c // 2) % srcb_c == 0   # j-blocks must not straddle halves
        self.zc_sub = self.chalf
        self.dt = mybir.dt.float16
        self.npdt = np.float16
        self.dt8 = mybir.dt.float8e4
        self.np8 = ml_dtypes.float8_e4m3


def _ru(x, m):
    return (x + m - 1) // m * m


def _wrap_idx(flat):
    """[n] int16 -> [128, n/16] wrap (16-partition, replicated x8)."""
    n = flat.shape[0]
    w = flat.astype(np.int16).reshape(n // 16, 16).T
    return np.tile(w, (8, 1))


# ---------------------------------------------------------------------------
# host-side structure + array prep
# ---------------------------------------------------------------------------

def _emissions(dstrel_all, T, W, nwin):
    """Per-tile strip ranges (union over cores).  dstrel_all: [NC, T*128]
    with -1e9 for pad.  Returns list of (t, wlo, nw)."""
    Dw = dstrel_all.reshape(NCORES, T, P)
    valid = Dw.max(axis=2) >= 0                    # [NC, T]
    wlo_c = np.where(Dw >= 0, Dw, np.inf).min(axis=2) // W
    whi_c = np.where(Dw >= 0, Dw, -np.inf).max(axis=2) // W
    any_v = valid.any(axis=0)
    wlo = np.where(valid, wlo_c, np.inf).min(axis=0)
    whi = np.where(valid, whi_c, -np.inf).max(axis=0)
    out = []
    for t in range(T):
        if not any_v[t]:
            continue
        a = max(0, min(int(wlo[t]), nwin - 1))
        b = max(a, min(int(whi[t]), nwin - 1))
        out.append((t, a, b - a + 1))
    return out


def _build_p(dstrel, scl, ems, T, W):
    """P stream [128, ncols] fp16 for one core.  dstrel/scl: [T*128]."""
    if not ems:
        return np.zeros((P, W), np.float16)
    et = np.array([e[0] for e in ems for _ in range(e[2])])
    ew = np.array([e[1] + i for e in ems for i in range(e[2])])
    D = dstrel.reshape(T, P)[et]                     # [ne, 128]
    S = scl.reshape(T, P)[et]
    del S
    tgt = (ew[:, None] * W + np.arange(W)[None, :])  # [ne, W]
    blk = (D[:, :, None] == tgt[:, None, :])         # binary (scale applied
    return np.ascontiguousarray(                     # post-agg on device)
        blk.transpose(1, 0, 2).reshape(P, -1)).astype(np.float16)


class L1Pass:
    """Layer-1 aggregation: host-packed X stream + P strips, no gathers."""

    def __init__(self, name, nloc, sbn, W):
        self.name, self.nloc, self.sbn, self.W = name, nloc, sbn, W
        self.nsb = -(-nloc // sbn)
        self.sb_nodes = [min(sbn, nloc - s * sbn) for s in range(self.nsb)]
        self.sb_len = None       # [nsb] padded slots per superblock
        self.offs = None
        self.etot = 0
        self.emits = None        # per sb: list of (t_local, wlo, nw)
        self.p_off = None        # per sb: P-stream col offset of first emit
        self.p_cols = 0
        self.src_order = None    # per core [etot] int64 (for X pack)
        self.pstreams = None     # per core [128, p_cols] fp16


def prep_l1(name, src, dst_loc, scl_e, core_e, nloc, sbn, W):
    ap = L1Pass(name, nloc, sbn, W)
    nsb = ap.nsb
    per_core = []
    counts = np.zeros((NCORES, nsb), np.int64)
    for k in range(NCORES):
        m = core_e == k
        s, d, sc = src[m], dst_loc[m], scl_e[m]
        order = np.argsort(d, kind="stable")
        s, d, sc = s[order], d[order], sc[order]
        sb = d // sbn
        counts[k] = np.bincount(sb, minlength=nsb)
        per_core.append((s, d, sc, sb))
    sb_len = _ru(counts.max(axis=0), P)
    offs = np.concatenate([[0], np.cumsum(sb_len)]).astype(np.int64)
    etot = int(offs[-1])
    ap.sb_len, ap.offs, ap.etot = sb_len, offs, etot

    dstrel_all = np.full((NCORES, etot), -1.0e9, np.float64)
    src_order, scl_all = [], []
    for k in range(NCORES):
        s, d, sc, sb = per_core[k]
        start = np.concatenate([[0], np.cumsum(counts[k])])[:-1]
        pos = offs[sb] + (np.arange(len(s)) - start[sb])
        so = np.zeros(etot, np.int64)
        so[pos] = s
        sl = np.zeros(etot, np.float32)
        sl[pos] = sc
        dstrel_all[k, pos] = (d - sb * sbn).astype(np.float64)
        src_order.append(so)
        scl_all.append(sl)
    ap.src_order = src_order

    emits, p_off = [], []
    pc = 0
    for s in range(nsb):
        T = int(sb_len[s]) // P
        nwin = -(-ap.sb_nodes[s] // W)
        o = int(offs[s])
        ems = _emissions(dstrel_all[:, o:o + sb_len[s]], T, W, nwin)
        emits.append(ems)
        p_off.append(pc)
        pc += sum(e[2] for e in ems) * W
    pc = _ru(max(pc, W), XCH)
    ap.emits, ap.p_off, ap.p_cols = emits, p_off, pc

    pstreams = []
    for k in range(NCORES):
        parts = []
        for s in range(nsb):
            if not emits[s]:
                continue
            o = int(offs[s])
            L = int(sb_len[s])
            dr = np.where(dstrel_all[k, o:o + L] < -1.0, -1000.0,
                          dstrel_all[k, o:o + L])
            parts.append(_build_p(dr, scl_all[k][o:o + L], emits[s],
                                  L // P, W))
        full = np.zeros((P, pc), np.float16)
        if parts:
            ps = np.concatenate(parts, axis=1)
            full[:, :ps.shape[1]] = ps
        pstreams.append(full)
    ap.pstreams = pstreams
    return ap


class L2Pass:
    """Layer-2 aggregation: runtime gathers per (sb, src-block) run +
    shipped P strips."""

    def __init__(self, name, nloc, srcb, nblk, nsrc_rows, sbn, W):
        self.name, self.nloc, self.srcb, self.nblk = name, nloc, srcb, nblk
        self.nsrc_rows, self.sbn, self.W = nsrc_rows, sbn, W
        self.nsb = -(-nloc // sbn)
        self.sb_nodes = [min(sbn, nloc - s * sbn) for s in range(self.nsb)]
        self.run_L = None        # [nsb, nblk]
        self.offs = None
        self.etot = 0
        self.emits = None        # per sb: list of (j, t_run, wlo, nw)
        self.p_off = None
        self.p_cols = 0
        self.idx = None          # per core [128, etot/16] int16
        self.pstreams = None


def prep_l2(name, src, dst_loc, scl_e, core_e, nloc, srcb, nblk,
            nsrc_rows, sbn, W):
    ap = L2Pass(name, nloc, srcb, nblk, nsrc_rows, sbn, W)
    nsb = ap.nsb
    nruns = nsb * nblk
    per_core = []
    counts = np.zeros((NCORES, nruns), np.int64)
    for k in range(NCORES):
        m = core_e == k
        s, d, sc = src[m], dst_loc[m], scl_e[m]
        j = s // srcb
        sb = d // sbn
        order = np.lexsort((d, j, sb))
        s, d, sc, j, sb = s[order], d[order], sc[order], j[order], sb[order]
        rid = sb * nblk + j
        counts[k] = np.bincount(rid, minlength=nruns)
        per_core.append((s, d, sc, rid, sb))
    run_L = _ru(counts.max(axis=0), P)
    offs = np.concatenate([[0], np.cumsum(run_L)]).astype(np.int64)
    etot = int(offs[-1])
    ap.run_L, ap.offs, ap.etot = run_L.reshape(nsb, nblk), offs, etot

    dstrel_all = np.full((NCORES, etot), -1.0e9, np.float64)
    idxs, scl_all = [], []
    for k in range(NCORES):
        s, d, sc, rid, sb = per_core[k]
        start = np.concatenate([[0], np.cumsum(counts[k])])[:-1]
        pos = offs[rid] + (np.arange(len(s)) - start[rid])
        i16 = np.zeros(etot, np.int16)
        i16[pos] = (s - (s // srcb) * srcb).astype(np.int16)
        sl = np.zeros(etot, np.float32)
        sl[pos] = sc
        dstrel_all[k, pos] = (d - sb * sbn).astype(np.float64)
        idxs.append(_wrap_idx(i16))
        scl_all.append(sl)
    ap.idx = idxs

    emits, p_off = [], []
    pc = 0
    for s in range(nsb):
        nwin = -(-ap.sb_nodes[s] // W)
        sb_ems = []
        for j in range(nblk):
            r = s * nblk + j
            o = int(offs[r])
            L = int(ap.run_L[s, j])
            if L == 0:
                continue
            ems = _emissions(dstrel_all[:, o:o + L], L // P, W, nwin)
            sb_ems.extend((j, t, a, nw) for (t, a, nw) in ems)
        emits.append(sb_ems)
        p_off.append(pc)
        pc += sum(e[3] for e in sb_ems) * W
    pc = _ru(max(pc, W), XCH)
    ap.emits, ap.p_off, ap.p_cols = emits, p_off, pc

    # flatten all strips (pass order) for vectorized P build
    em_base, em_w = [], []
    for s in range(nsb):
        for (j, t, a, nw) in emits[s]:
            o = int(offs[s * nblk + j]) + t * P
            for wi in range(nw):
                em_base.append(o)
                em_w.append(a + wi)
    em_base = np.array(em_base, np.int64)
    em_w = np.array(em_w, np.int64)
    pstreams = []
    for k in range(NCORES):
        if len(em_base) == 0:
            pstreams.append(np.zeros((P, pc), np.float16))
            continue
        D = dstrel_all[k][em_base[:, None] + np.arange(P)[None, :]]
        D = np.where(D < -1.0, -1000.0, D)
        tgt = em_w[:, None] * W + np.arange(W)[None, :]
        blk = (D[:, :, None] == tgt[:, None, :])
        ps = blk.transpose(1, 0, 2).reshape(P, -1)
        full = np.zeros((P, pc), np.float16)
        full[:, :ps.shape[1]] = ps.astype(np.float16)
        pstreams.append(full)
    ap.pstreams = pstreams
    return ap


def prep_all(cfg, inputs):
    i64 = lambda a: np.asarray(a).astype(np.int64)
    e_src = i64(inputs["edge_src_customer"])
    e_dst = i64(inputs["edge_dst_article"])
    l_c = i64(inputs["label_customer"])
    l_a = i64(inputs["label_article"])

    cnt_a = np.bincount(e_dst, minlength=cfg.n_a)
    cnt_c = np.bincount(e_src, minlength=cfg.n_c)
    scl_a = (1.0 / np.maximum(cnt_a, 1.0)).astype(np.float32)
    scl_c = (1.0 / np.maximum(cnt_c, 1.0)).astype(np.float32)

    core_a = e_dst // cfg.apc          # A-pass owner (dst article)
    core_c = e_src // cfg.cpc          # C-pass owner (dst customer)
    dloc_c = e_src % cfg.cpc
    half = (dloc_c >= cfg.chalf).astype(np.int64)

    # ---- layer 1 (host-packed X, no gathers) ----
    pa1 = prep_l1("A1", e_src, e_dst % cfg.apc, scl_a[e_dst], core_a,
                  cfg.apc, cfg.sbn, cfg.w1a)
    pc1 = []
    for h in range(2):
        m = half == h
        pc1.append(prep_l1(f"C1{h}", e_dst[m], dloc_c[m] - h * cfg.chalf,
                           scl_c[e_src][m], core_c[m], cfg.chalf, cfg.sbn,
                           cfg.w1c))

    # ---- layer 2 (runtime gathers; customers relabeled half-major) ----
    rel_c = half * (cfg.n_c // 2) + (e_src // cfg.cpc) * cfg.chalf + \
        (dloc_c - half * cfg.chalf)
    pa2 = prep_l2("A2", rel_c, e_dst % cfg.apc, scl_a[e_dst], core_a,
                  cfg.apc, cfg.srcb_c, cfg.nblk_c, cfg.n_c, cfg.sbn,
                  cfg.w2a)
    pc2 = []
    for h in range(2):
        m = half == h
        pc2.append(prep_l2(f"C2{h}", e_dst[m], dloc_c[m] - h * cfg.chalf,
                           scl_c[e_src][m], core_c[m], cfg.chalf,
                           cfg.srcb_a, cfg.nblk_a, cfg.n_a, cfg.sbn,
                           cfg.w2c))

    # ---- decoder labels: partition by customer core, group (sub, ablk) ----
    core_l = l_c // cfg.cpc
    sub_l = (l_c % cfg.cpc) // cfg.zc_sub
    ablk_l = l_a // cfg.srcb_a
    gid = sub_l * cfg.nblk_a + ablk_l
    ngrp = 2 * cfg.nblk_a
    gcounts = np.zeros((NCORES, ngrp), np.int64)
    per_core_lbl = []
    for k in range(NCORES):
        m = core_l == k
        lc, la, g, orig = l_c[m], l_a[m], gid[m], np.nonzero(m)[0]
        order = np.argsort(g, kind="stable")
        lc, la, g, orig = lc[order], la[order], g[order], orig[order]
        gcounts[k] = np.bincount(g, minlength=ngrp)
        per_core_lbl.append((lc, la, g, orig))
    grp_L = _ru(gcounts.max(axis=0), P)
    goffs = np.concatenate([[0], np.cumsum(grp_L)]).astype(np.int64)
    ld_pad = int(goffs[-1])

    # per-label customer offset within its sub (for Sel blocks), sorted by
    # customer within each group so the Uc side is a dense Sel-matmul
    dec_idx_a, out_pos = [], []
    lcrel_all = np.full((NCORES, ld_pad), -1.0e9, np.float64)
    for k in range(NCORES):
        lc, la, g, orig = per_core_lbl[k]
        lcl = lc % cfg.cpc
        lcr = lcl - (lcl // cfg.zc_sub) * cfg.zc_sub
        order = np.lexsort((lcr, g))
        lc, la, g, orig, lcr = (lc[order], la[order], g[order],
                                orig[order], lcr[order])
        gstart = np.concatenate([[0], np.cumsum(gcounts[k])])[:-1]
        pos = goffs[g] + (np.arange(len(lc)) - gstart[g])
        ia = np.zeros(ld_pad, np.int16)
        po = np.full(ld_pad, -1, np.int64)
        ia[pos] = (la - (la // cfg.srcb_a) * cfg.srcb_a).astype(np.int16)
        po[pos] = orig
        lcrel_all[k, pos] = lcr.astype(np.float64)
        dec_idx_a.append(_wrap_idx(ia))
        out_pos.append(po)

    # Sel emissions (union over cores) + per-core fp8 Sel streams
    subw = -(-cfg.zc_sub // P)             # cust windows per padded sub
    sel_ems = []                           # per group: list of (t, wlo, nw)
    for gi in range(ngrp):
        o, L = int(goffs[gi]), int(grp_L.reshape(-1)[gi])
        if L == 0:
            sel_ems.append([])
            continue
        sel_ems.append(_emissions(lcrel_all[:, o:o + L], L // P, P, subw))
    em_base, em_w = [], []
    for gi in range(ngrp):
        o = int(goffs[gi])
        for (t, a, nw) in sel_ems[gi]:
            for wi in range(nw):
                em_base.append(o + t * P)
                em_w.append(a + wi)
    em_base = np.array(em_base, np.int64)
    em_w = np.array(em_w, np.int64)
    sel_cols = _ru(max(len(em_base) * P, P), XCH)
    sels = []
    for k in range(NCORES):
        D = lcrel_all[k][em_base[:, None] + np.arange(P)[None, :]]
        D = np.where(D < -1.0, -1000.0, D)
        tgt = em_w[:, None] * P + np.arange(P)[None, :]
        blk = (D[:, :, None] == tgt[:, None, :])   # [ne, label, cust]
        ps = blk.transpose(2, 0, 1).reshape(P, -1)  # part=cust, col=label
        full = np.zeros((P, sel_cols), np.float16)
        full[:, :ps.shape[1]] = ps
        sels.append(full)

    dec = dict(grp_L=grp_L.reshape(2, cfg.nblk_a), goffs=goffs,
               ld_pad=ld_pad, idx_a=dec_idx_a, out_pos=out_pos,
               sel_ems=sel_ems, sel_cols=sel_cols, sels=sels, subw=subw)
    return pa1, pc1, pa2, pc2, dec


# ---------------------------------------------------------------------------
# kernel builder
# ---------------------------------------------------------------------------

F32 = mybir.dt.float32


def build_nc(cfg, pa1, pc1, pa2, pc2, dec):
    DT = cfg.dt
    nc = bacc.Bacc("TRN2", target_bir_lowering=False, debug=False,
                   num_devices=NCORES, num_swdge_queues=4)
    qctr = [0]

    def next_q():
        # 4-queue rotation lets the 8 GpSimd Q7 cores overlap descriptor
        # generation across gathers (~2x).  CoreSim's per-sem-lane queue
        # lock can't be satisfied under scheduler reordering; hardware
        # tolerates mixed queues per lane (sems are plain counters).
        if SIM_SINGLE_QUEUE:
            return 0
        qctr[0] = (qctr[0] + 1) % 4
        return qctr[0]

    ei = lambda n, s, d: nc.dram_tensor(n, s, d, kind="ExternalInput")
    xaT = ei("xaT", [P, cfg.apc], DT)
    xcT = ei("xcT", [P, cfg.cpc], DT)
    DT8 = cfg.dt8
    xA1 = ei("xA1", [P, pa1.etot], DT)
    xC1 = [ei(f"xC1{h}", [P, pc1[h].etot], DT) for h in range(2)]
    pA1 = ei("pA1", [P, pa1.p_cols], DT8)
    pC1 = [ei(f"pC1{h}", [P, pc1[h].p_cols], DT8) for h in range(2)]
    pA2 = ei("pA2", [P, pa2.p_cols], DT8)
    pC2 = [ei(f"pC2{h}", [P, pc2[h].p_cols], DT8) for h in range(2)]
    iA2 = ei("iA2", [P, pa2.etot // 16], mybir.dt.int16)
    iC2 = [ei(f"iC2{h}", [P, pc2[h].etot // 16], mybir.dt.int16)
           for h in range(2)]
    dec_idx_a = ei("dec_idx_a", [P, dec["ld_pad"] // 16], mybir.dt.int16)
    selD = ei("selD", [P, dec["sel_cols"]], cfg.dt8)
    sclA = ei("sclA", [1, cfg.apc], DT)
    sclC = ei("sclC", [1, cfg.cpc], DT)

    wnames = ["W_msg1_ca", "W_self1_a", "W_msg1_ac", "W_self1_c",
              "W_msg2_ca", "W_self2_a", "W_msg2_ac", "W_self2_c",
              "Wd1c", "Wd1a"]
    wts = {n: ei(n, [P, P], DT) for n in wnames}
    w2rep = ei("w2rep", [P, DGCH], DT)
    bnames = ["b1_a", "b1_c", "b2_a", "b2_c",
              "bn_gamma_c", "bn_beta_c", "bn_gamma_a", "bn_beta_a",
              "b_dec1", "b_dec2c"]
    bis = {n: ei(n, [P, 1], F32) for n in bnames}

    ldT = dec["ld_pad"] // P
    y_out = nc.dram_tensor("y", [P, ldT], F32, kind="ExternalOutput")
    rg = [list(range(NCORES))]

    with tile.TileContext(nc) as tc:
        with (
            tc.tile_pool(name="dramp", bufs=1, space="DRAM") as dramp,
            tc.tile_pool(name="const", bufs=1) as cs,
        ):
            ha_own = dramp.tile([cfg.apc, P], DT)
            ha_full = dramp.tile([cfg.n_a, P], DT, addr_space="Shared")
            hc_own = [dramp.tile([cfg.chalf, P], DT, name=f"hc_own{h}")
                      for h in range(2)]
            hc_full = [dramp.tile([cfg.n_c // 2, P], DT,
                                  addr_space="Shared", name=f"hc_full{h}")
                       for h in range(2)]
            ua_own = dramp.tile([cfg.apc, P], DT)
            ua_full = dramp.tile([cfg.n_a, P], DT, addr_space="Shared")
            uc_pad = dramp.tile([2 * dec["subw"] * P, P], DT)
            haT_d = dramp.tile([P, cfg.apc], DT)
            hcT_d = dramp.tile([P, cfg.cpc], DT)
            zaT_d = dramp.tile([P, cfg.apc], DT)
            zcT_d = dramp.tile([P, cfg.cpc], DT)
            stats_in_a = dramp.tile([P, 2], F32)
            stats_out_a = dramp.tile([P, 2], F32, addr_space="Shared")
            stats_in_c = dramp.tile([P, 2], F32)
            stats_out_c = dramp.tile([P, 2], F32, addr_space="Shared")

            ident = cs.tile([P, P], DT)
            make_identity(nc, ident[:])
            w_sb = {n: cs.tile([P, P], DT, name=f"w_{n}") for n in wnames}
            for n in wnames:
                nc.sync.dma_start(out=w_sb[n][:], in_=wts[n][:])
            w2r_sb = cs.tile([P, DGCH], DT)
            nc.sync.dma_start(out=w2r_sb[:], in_=w2rep[:])
            b_sb = {n: cs.tile([P, 1], F32, name=f"b_{n}") for n in bnames}
            for n in bnames:
                nc.sync.dma_start(out=b_sb[n][:], in_=bis[n][:])
            stats_sb = cs.tile([P, 4], F32)
            nc.vector.memset(stats_sb[:], 0.0)
            ones1 = cs.tile([1, P], DT)
            nc.vector.memset(ones1[:], 1.0)

            # ------------- per-dst 1/cnt replicated across partitions ---
            def build_sclrep(scl_dram, off, nloc, sbp, psp):
                srow = sbp.tile([1, nloc], DT, tag="srow", name="srow",
                                bufs=1)
                nc.sync.dma_start(out=srow[:], in_=scl_dram[:, off:off + nloc])
                srep = sbp.tile([P, _ru(nloc, BANK)], DT, tag="srep",
                                name="srep", bufs=1)
                for c0 in range(0, nloc, BANK):
                    bw = min(BANK, nloc - c0)
                    sp = psp.tile([P, BANK], F32, tag="wps", name="sclps",
                                  bufs=2)
                    nc.tensor.matmul(sp[:, :bw], lhsT=ones1[:],
                                     rhs=srow[:, c0:c0 + bw], start=True,
                                     stop=True, skip_group_check=True)
                    nc.scalar.copy(srep[:, c0:c0 + bw], sp[:, :bw])
                return srep

            # ------------- shared W-stage (per superblock) -------------
            def w_stage_sb(meanT_sb, c0, cw, selfT_dram, self_off, wmsg,
                           wself, bias_col, relu, outT_dram, outT_off,
                           rows_dram, rows_off, stats_cols, sbp, psp):
                """One 512-col (max) chunk loop over [c0, c0+cw)."""
                for b0 in range(0, cw, BANK):
                    bw = min(BANK, cw - b0)
                    sT = sbp.tile([P, BANK], DT, tag="wself", name="wselfT",
                                  bufs=3)
                    nc.sync.dma_start(
                        out=sT[:, :bw],
                        in_=selfT_dram[:, self_off + c0 + b0:
                                       self_off + c0 + b0 + bw])
                    psum = psp.tile([P, BANK], F32, tag="wps", name="wps",
                                    bufs=2)
                    nc.tensor.matmul(psum[:, :bw], lhsT=wmsg,
                                     rhs=meanT_sb[:, c0 + b0:c0 + b0 + bw],
                                     start=True, stop=False,
                                     skip_group_check=True)
                    nc.tensor.matmul(psum[:, :bw], lhsT=wself,
                                     rhs=sT[:, :bw], start=False, stop=True,
                                     skip_group_check=True)
                    oT = sbp.tile([P, BANK], DT, tag="woT", name="woT",
                                  bufs=3)
                    nc.scalar.activation(
                        oT[:, :bw], psum[:, :bw],
                        mybir.ActivationFunctionType.Relu if relu
                        else mybir.ActivationFunctionType.Identity,
                        bias=bias_col[:], scale=1.0)
                    nc.sync.dma_start(
                        out=outT_dram[:, outT_off + c0 + b0:
                                      outT_off + c0 + b0 + bw],
                        in_=oT[:, :bw])
                    if stats_cols is not None:
                        si, sj = stats_cols
                        part = sbp.tile([P, 1], F32, tag="wst1", name="wst1",
                                        bufs=2)
                        nc.vector.reduce_sum(part[:], oT[:, :bw],
                                             mybir.AxisListType.X)
                        nc.vector.tensor_add(stats_sb[:, si:si + 1],
                                             stats_sb[:, si:si + 1],
                                             part[:])
                        trash = sbp.tile([P, BANK], F32, tag="wtrash",
                                         name="wtrash", bufs=2)
                        part2 = sbp.tile([P, 1], F32, tag="wst2",
                                         name="wst2", bufs=2)
                        nc.scalar.activation(
                            trash[:, :bw], oT[:, :bw],
                            mybir.ActivationFunctionType.Square,
                            accum_out=part2[:])
                        nc.vector.tensor_add(stats_sb[:, sj:sj + 1],
                                             stats_sb[:, sj:sj + 1],
                                             part2[:])
                    if rows_dram is not None:
                        _emit_rows(oT, bw, rows_dram, rows_off + c0 + b0,
                                   sbp, psp)

            def _emit_rows(srcT_sb, cw, rows_dram, row_base, sbp, psp):
                """Transpose [128, cw<=512] -> cw rows in DRAM, batched."""
                rows = sbp.tile([P, BANK], DT, tag="rows", name="rows",
                                bufs=3)
                nb = -(-cw // P)
                for i in range(nb):
                    b0 = i * P
                    bw = min(P, cw - b0)
                    tp = psp.tile([P, P], DT, tag="tps", name="tps", bufs=2)
                    nc.tensor.transpose(tp[:bw, :], srcT_sb[:, b0:b0 + bw],
                                        ident[:])
                    nc.scalar.copy(rows[:bw, i * P:(i + 1) * P], tp[:bw, :])
                out_ap = rows_dram[row_base:row_base + cw, :]
                if cw == nb * P:
                    out_ap = out_ap.rearrange("(c p) f -> p c f", p=P)
                    nc.sync.dma_start(out=out_ap, in_=rows[:, :cw]
                                      .rearrange("p (c f) -> p c f", f=P))
                else:
                    for i in range(nb):
                        b0 = i * P
                        bw = min(P, cw - b0)
                        nc.sync.dma_start(
                            out=rows_dram[row_base + b0:row_base + b0 + bw,
                                          :],
                            in_=rows[:bw, i * P:i * P + P])

            # ------------- L1 aggregation (streamed, no gathers) --------
            def agg_l1(ps, x_d, p_d, meanT_sb, srep, sbp, psp, wargs):
                W = ps.W
                xoff = 0
                poff = 0
                for s in range(ps.nsb):
                    nodes = ps.sb_nodes[s]
                    L = int(ps.sb_len[s])
                    T = L // P
                    psum = psp.tile([P, cfg.sbn], F32, tag="aggps",
                                    name="aggps", bufs=2)
                    ems = ps.emits[s]
                    # bank bookkeeping
                    firsts, lasts, banks = {}, {}, set()
                    seq = []
                    for (t, a, nw) in ems:
                        for wi in range(nw):
                            w = a + wi
                            b = (w * W) // BANK
                            seq.append((t, w, b))
                            banks.add(b)
                    for i, (t, w, b) in enumerate(seq):
                        if b not in firsts:
                            firsts[b] = i
                        lasts[b] = i
                    # X stream chunks covering this sb
                    x_tiles = {}
                    for c0 in range(0, L, XCH):
                        cl = min(XCH, L - c0)
                        xt = sbp.tile([P, XCH], DT, tag="xs", name="xs",
                                      bufs=3)
                        nc.sync.dma_start(
                            out=xt[:, :cl],
                            in_=x_d[:, xoff + c0:xoff + c0 + cl])
                        x_tiles[c0 // XCH] = xt
                    # P stream chunks
                    pcols = sum(e[2] for e in ems) * W
                    p_tiles = {}
                    for c0 in range(0, pcols, XCH):
                        cl = min(XCH, pcols - c0)
                        pt = sbp.tile([P, XCH], cfg.dt8, tag="pstr",
                                      name="pstr", bufs=3)
                        nc.sync.dma_start(
                            out=pt[:, :cl],
                            in_=p_d[:, poff + c0:poff + c0 + cl])
                        p_tiles[c0 // XCH] = pt
                    pcur = 0
                    for i, (t, w, b) in enumerate(seq):
                        xt = x_tiles[(t * P) // XCH]
                        xsl = xt[:, (t * P) % XCH:(t * P) % XCH + P]
                        pt = p_tiles[pcur // XCH]
                        psl = pt[:, pcur % XCH:pcur % XCH + W]
                        nc.tensor.matmul(
                            psum[:, w * W:(w + 1) * W], lhsT=xsl, rhs=psl,
                            start=(firsts[b] == i), stop=(lasts[b] == i),
                            skip_group_check=True)
                        pcur += W
                    _sb_finish(psum, banks, nodes, s, meanT_sb, srep)
                    xoff += L
                    poff += pcols
                    w_stage_sb(meanT_sb, s * cfg.sbn, nodes, *wargs,
                               sbp=sbp, psp=psp)

            def _sb_finish(psum, banks, nodes, s, meanT_sb, srep):
                """meanT = psum * sclrep (DVE) + memset uncovered banks."""
                nbank = -(-nodes // BANK)
                for b in range(nbank):
                    a = b * BANK
                    e = min((b + 1) * BANK, nodes)
                    dst = meanT_sb[:, s * cfg.sbn + a:s * cfg.sbn + e]
                    if b in banks:
                        nc.vector.tensor_tensor(
                            out=dst, in0=psum[:, a:e],
                            in1=srep[:, s * cfg.sbn + a:s * cfg.sbn + e],
                            op=mybir.AluOpType.mult)
                    else:
                        nc.vector.memset(dst, 0.0)

            # ------------- L2 aggregation (gather + shipped P) ----------
            def agg_l2(ps, tables, idx_d, p_d, meanT_sb, srep, sbp, psp,
                       wargs):
                """tables: list of (j_lo, j_hi, dram_tile, row_off, rows)."""
                W = ps.W
                poff = 0
                for s in range(ps.nsb):
                    nodes = ps.sb_nodes[s]
                    psum = psp.tile([P, cfg.sbn], F32, tag="aggps",
                                    name="aggps", bufs=2)
                    ems = ps.emits[s]
                    firsts, lasts, banks = {}, {}, set()
                    seq = []
                    for (j, t, a, nw) in ems:
                        for wi in range(nw):
                            w = a + wi
                            b = (w * W) // BANK
                            seq.append((j, t, w, b))
                            banks.add(b)
                    for i, (j, t, w, b) in enumerate(seq):
                        if b not in firsts:
                            firsts[b] = i
                        lasts[b] = i
                    # per-run gathers
                    x_runs = {}
                    rmax = int(ps.run_L.max())
                    for j in range(ps.nblk):
                        L = int(ps.run_L[s, j])
                        if L == 0:
                            continue
                        o = int(ps.offs[s * ps.nblk + j])
                        idx_sb = sbp.tile([P, rmax // 16], mybir.dt.int16,
                                          tag="gidx", name="gidx", bufs=4)
                        nc.sync.dma_start(
                            out=idx_sb[:, :L // 16],
                            in_=idx_d[:, o // 16:(o + L) // 16])
                        xg = sbp.tile([P, rmax // P, P], DT, tag="gx",
                                      name="gx", bufs=6)
                        done = False
                        for (jlo, jhi, tbl, roff, trows) in tables:
                            if jlo <= j <= jhi:
                                rel = j * ps.srcb - roff
                                blk_rows = min(ps.srcb, trows - rel)
                                for c0 in range(0, L, 1024):
                                    cl = min(1024, L - c0)
                                    nc.gpsimd.dma_gather(
                                        xg[:, c0 // P:(c0 + cl) // P, :],
                                        tbl[rel:rel + blk_rows, :],
                                        idx_sb[:, c0 // 16:(c0 + cl) // 16],
                                        cl, cl, P, queue_num=next_q())
                                done = True
                                break
                        assert done
                        x_runs[j] = xg
                    # P stream chunks
                    pcols = sum(e[3] for e in ems) * W
                    p_tiles = {}
                    for c0 in range(0, pcols, XCH):
                        cl = min(XCH, pcols - c0)
                        pt = sbp.tile([P, XCH], cfg.dt8, tag="pstr",
                                      name="pstr", bufs=3)
                        nc.sync.dma_start(
                            out=pt[:, :cl],
                            in_=p_d[:, poff + c0:poff + c0 + cl])
                        p_tiles[c0 // XCH] = pt
                    pcur = 0
                    for i, (j, t, w, b) in enumerate(seq):
                        xg = x_runs[j]
                        pt = p_tiles[pcur // XCH]
                        psl = pt[:, pcur % XCH:pcur % XCH + W]
                        nc.tensor.matmul(
                            psum[:, w * W:(w + 1) * W], lhsT=xg[:, t, :],
                            rhs=psl, start=(firsts[b] == i),
                            stop=(lasts[b] == i), skip_group_check=True)
                        pcur += W
                    _sb_finish(psum, banks, nodes, s, meanT_sb, srep)
                    poff += pcols
                    w_stage_sb(meanT_sb, s * cfg.sbn, nodes, *wargs,
                               sbp=sbp, psp=psp)

            # ================= layer 1 =================
            with (
                tc.tile_pool(name="l1a", bufs=1) as sbp,
                tc.tile_pool(name="l1ap", bufs=1, space="PSUM") as psp,
                tc.tile_pool(name="l1am", bufs=1) as mp,
            ):
                meanT = mp.tile([P, pa1.nsb * cfg.sbn], DT, name="meanTA")
                srep = build_sclrep(sclA, 0, cfg.apc, sbp, psp)
                agg_l1(pa1, xA1, pA1, meanT, srep, sbp, psp,
                       (xaT, 0, w_sb["W_msg1_ca"][:], w_sb["W_self1_a"][:],
                        b_sb["b1_a"], True, haT_d, 0, ha_own, 0, None))
            nc.gpsimd.collective_compute(
                "AllGather", mybir.AluOpType.bypass, replica_groups=rg,
                ins=[ha_own[:]], outs=[ha_full[:]])

            for h in range(2):
                with (
                    tc.tile_pool(name=f"l1c{h}", bufs=1) as sbp,
                    tc.tile_pool(name=f"l1cp{h}", bufs=1,
                                 space="PSUM") as psp,
                    tc.tile_pool(name=f"l1cm{h}", bufs=1) as mp,
                ):
                    meanT = mp.tile([P, pc1[h].nsb * cfg.sbn], DT,
                                    name="meanTC")
                    srep = build_sclrep(sclC, h * cfg.chalf, cfg.chalf,
                                        sbp, psp)
                    agg_l1(pc1[h], xC1[h], pC1[h], meanT, srep, sbp, psp,
                           (xcT, h * cfg.chalf, w_sb["W_msg1_ac"][:],
                            w_sb["W_self1_c"][:], b_sb["b1_c"], True,
                            hcT_d, h * cfg.chalf, hc_own[h], 0, None))
                nc.gpsimd.collective_compute(
                    "AllGather", mybir.AluOpType.bypass, replica_groups=rg,
                    ins=[hc_own[h][:]], outs=[hc_full[h][:]])

            # ================= layer 2 =================
            half_rows = cfg.n_c // 2
            nj_half = half_rows // cfg.srcb_c
            a2_tables = [(0, nj_half - 1, hc_full[0], 0, half_rows),
                         (nj_half, cfg.nblk_c - 1, hc_full[1], half_rows,
                          half_rows)]
            with (
                tc.tile_pool(name="l2a", bufs=1) as sbp,
                tc.tile_pool(name="l2ap", bufs=1, space="PSUM") as psp,
                tc.tile_pool(name="l2am", bufs=1) as mp,
            ):
                meanT = mp.tile([P, pa2.nsb * cfg.sbn], DT, name="meanTA2")
                srep = build_sclrep(sclA, 0, cfg.apc, sbp, psp)
                agg_l2(pa2, a2_tables, iA2, pA2, meanT, srep, sbp, psp,
                       (haT_d, 0, w_sb["W_msg2_ca"][:], w_sb["W_self2_a"][:],
                        b_sb["b2_a"], False, zaT_d, 0, None, 0, (0, 1)))
            nc.sync.dma_start(out=stats_in_a[:], in_=stats_sb[:, 0:2])
            nc.gpsimd.collective_compute(
                "AllReduce", mybir.AluOpType.add, replica_groups=rg,
                ins=[stats_in_a[:]], outs=[stats_out_a[:]])
            c2_tables = [(0, cfg.nblk_a - 1, ha_full, 0, cfg.n_a)]
            for h in range(2):
                with (
                    tc.tile_pool(name=f"l2c{h}", bufs=1) as sbp,
                    tc.tile_pool(name=f"l2cp{h}", bufs=1,
                                 space="PSUM") as psp,
                    tc.tile_pool(name=f"l2cm{h}", bufs=1) as mp,
                ):
                    meanT = mp.tile([P, pc2[h].nsb * cfg.sbn], DT,
                                    name="meanTC2")
                    srep = build_sclrep(sclC, h * cfg.chalf, cfg.chalf,
                                        sbp, psp)
                    agg_l2(pc2[h], c2_tables, iC2[h], pC2[h], meanT, srep,
                           sbp, psp,
                           (hcT_d, h * cfg.chalf, w_sb["W_msg2_ac"][:],
                            w_sb["W_self2_c"][:], b_sb["b2_c"], False,
                            zcT_d, h * cfg.chalf, None, 0, (2, 3)))

            # ================= BN + U tables =================
            with (
                tc.tile_pool(name="bn", bufs=1) as sbp,
                tc.tile_pool(name="bnp", bufs=1, space="PSUM") as psp,
            ):
                nc.sync.dma_start(out=stats_in_c[:], in_=stats_sb[:, 2:4])
                nc.gpsimd.collective_compute(
                    "AllReduce", mybir.AluOpType.add, replica_groups=rg,
                    ins=[stats_in_c[:]], outs=[stats_out_c[:]])
                st = sbp.tile([P, 4], F32)
                nc.sync.dma_start(out=st[:, 0:2], in_=stats_out_a[:])
                nc.sync.dma_start(out=st[:, 2:4], in_=stats_out_c[:])

                def bn_coeff(si, sj, n, gamma, beta, tagp):
                    mu = sbp.tile([P, 1], F32, name=f"mu{tagp}")
                    nc.vector.tensor_scalar_mul(mu[:], st[:, si:si + 1],
                                                1.0 / n)
                    msq = sbp.tile([P, 1], F32, name=f"msq{tagp}")
                    nc.vector.tensor_scalar_mul(msq[:], st[:, sj:sj + 1],
                                                1.0 / n)
                    mu2 = sbp.tile([P, 1], F32, name=f"mu2{tagp}")
                    nc.vector.tensor_mul(mu2[:], mu[:], mu[:])
                    var = sbp.tile([P, 1], F32, name=f"var{tagp}")
                    nc.vector.tensor_sub(var[:], msq[:], mu2[:])
                    nc.vector.tensor_scalar_add(var[:], var[:], BN_EPS)
                    sd = sbp.tile([P, 1], F32, name=f"sd{tagp}")
                    nc.scalar.activation(sd[:], var[:],
                                         mybir.ActivationFunctionType.Sqrt)
                    rstd = sbp.tile([P, 1], F32, name=f"rstd{tagp}")
                    nc.vector.reciprocal(rstd[:], sd[:])
                    scl = sbp.tile([P, 1], F32, name=f"scl{tagp}")
                    nc.vector.tensor_mul(scl[:], b_sb[gamma][:], rstd[:])
                    mg = sbp.tile([P, 1], F32, name=f"mg{tagp}")
                    nc.vector.tensor_mul(mg[:], mu[:], scl[:])
                    bia = sbp.tile([P, 1], F32, name=f"bia{tagp}")
                    nc.vector.tensor_sub(bia[:], b_sb[beta][:], mg[:])
                    return scl, bia

                scl_a_c, bia_a_c = bn_coeff(0, 1, cfg.n_a, "bn_gamma_a",
                                            "bn_beta_a", "a")

                def bn_u(nloc, zT_dram, scl, bia, w1half, ubias, rows_dram):
                    bn_u_sub(nloc, zT_dram, 0, scl, bia, w1half, ubias,
                             rows_dram, 0)

                def bn_u_sub(nloc, zT_dram, z_off, scl, bia, w1half, ubias,
                             rows_dram, r_off):
                    for c0 in range(0, nloc, BANK):
                        cw = min(BANK, nloc - c0)
                        zT = sbp.tile([P, BANK], DT, tag="bnz", name="bnz",
                                      bufs=3)
                        nc.sync.dma_start(
                            out=zT[:, :cw],
                            in_=zT_dram[:, z_off + c0:z_off + c0 + cw])
                        bnT = sbp.tile([P, BANK], DT, tag="bnt", name="bnt",
                                       bufs=3)
                        nc.vector.tensor_scalar(
                            out=bnT[:, :cw], in0=zT[:, :cw], scalar1=scl[:],
                            scalar2=bia[:], op0=mybir.AluOpType.mult,
                            op1=mybir.AluOpType.add)
                        ups = psp.tile([P, BANK], F32, tag="ups",
                                       name="ups", bufs=2)
                        nc.tensor.matmul(ups[:, :cw], lhsT=w1half,
                                         rhs=bnT[:, :cw], start=True,
                                         stop=True, skip_group_check=True)
                        uT = sbp.tile([P, BANK], DT, tag="uT", name="uT",
                                      bufs=3)
                        nc.scalar.activation(
                            uT[:, :cw], ups[:, :cw],
                            mybir.ActivationFunctionType.Identity,
                            bias=ubias[:] if ubias is not None else 0.0,
                            scale=1.0)
                        _emit_rows(uT, cw, rows_dram, r_off + c0, sbp, psp)

                bn_u(cfg.apc, zaT_d, scl_a_c, bia_a_c, w_sb["Wd1a"][:],
                     None, ua_own)
                nc.gpsimd.collective_compute(
                    "AllGather", mybir.AluOpType.bypass, replica_groups=rg,
                    ins=[ua_own[:]], outs=[ua_full[:]])
                scl_c_c, bia_c_c = bn_coeff(2, 3, cfg.n_c, "bn_gamma_c",
                                            "bn_beta_c", "c")
                zpad = sbp.tile([P, P], DT, name="zpad")
                nc.vector.memset(zpad[:], 0.0)
                for sub in range(2):
                    bn_u_sub(cfg.zc_sub, zcT_d, sub * cfg.zc_sub, scl_c_c,
                             bia_c_c, w_sb["Wd1c"][:], b_sb["b_dec1"],
                             uc_pad, sub * dec["subw"] * P)
                    npad = dec["subw"] * P - cfg.zc_sub
                    if npad > 0:
                        nc.sync.dma_start(
                            out=uc_pad[sub * dec["subw"] * P + cfg.zc_sub:
                                       (sub + 1) * dec["subw"] * P, :],
                            in_=zpad[:npad, :])

            # ================= decoder =================
            grp_L = dec["grp_L"]
            goffs = dec["goffs"]
            subw = dec["subw"]
            with (
                tc.tile_pool(name="dec", bufs=1) as sbp,
                tc.tile_pool(name="decp", bufs=1, space="PSUM") as psp,
            ):
                ysb = sbp.tile([P, ldT], F32, name="ysb")
                scur = 0          # Sel stream cursor (cols)
                s_tiles = {}

                def sel_chunk(c0):
                    ci = c0 // XCH
                    if ci not in s_tiles:
                        st = sbp.tile([P, XCH], cfg.dt8, tag="sel",
                                      name="sel", bufs=3)
                        nc.sync.dma_start(
                            out=st[:], in_=selD[:, ci * XCH:(ci + 1) * XCH])
                        s_tiles[ci] = st
                    return s_tiles[ci]

                for sub in range(2):
                    ucres = sbp.tile([P, subw * P], DT, tag="ucres",
                                     name="ucres", bufs=1)
                    nc.sync.dma_start(
                        out=ucres[:].rearrange("p (w f) -> p w f", f=P),
                        in_=uc_pad[sub * subw * P:(sub + 1) * subw * P, :]
                        .rearrange("(w p) f -> p w f", p=P))
                    for ab in range(cfg.nblk_a):
                        gi = sub * cfg.nblk_a + ab
                        L = int(grp_L[sub, ab])
                        o = int(goffs[gi])
                        ua_rows = min(cfg.srcb_a,
                                      cfg.n_a - ab * cfg.srcb_a)
                        ems = dec["sel_ems"][gi]
                        by_tile = {}
                        for (t, a, nw) in ems:
                            by_tile[t] = (a, nw)
                        for c0 in range(0, L, DGCH):
                            cl = min(DGCH, L - c0)
                            oc = o + c0
                            ctn = cl // P
                            ixa = sbp.tile([P, DGCH // 16], mybir.dt.int16,
                                           tag="dixa", name="dixa", bufs=4)
                            nc.sync.dma_start(
                                out=ixa[:, :cl // 16],
                                in_=dec_idx_a[:, oc // 16:(oc + cl) // 16])
                            uag = sbp.tile([P, DGCH // P, P], DT, tag="dua",
                                           name="dua", bufs=4)
                            for g0 in range(0, cl, 1024):
                                gl = min(1024, cl - g0)
                                nc.gpsimd.dma_gather(
                                    uag[:, g0 // P:(g0 + gl) // P, :],
                                    ua_full[ab * cfg.srcb_a:
                                            ab * cfg.srcb_a + ua_rows, :],
                                    ixa[:, g0 // 16:(g0 + gl) // 16],
                                    gl, gl, P, queue_num=next_q())
                            rel = sbp.tile([P, DGCH], DT, tag="drel",
                                           name="drel", bufs=3)
                            # Uc via Sel-matmul + Ua folded in via identity
                            # matmul; relu straight from psum (4 tiles/bank)
                            for q0 in range(0, ctn, 4):
                                qn = min(4, ctn - q0)
                                dps = psp.tile([P, 4 * P], F32, tag="dps",
                                               name="dps", bufs=2)
                                mm = []
                                for ti in range(qn):
                                    t = (c0 // P) + q0 + ti
                                    if t in by_tile:
                                        a, nw = by_tile[t]
                                        for wi in range(nw):
                                            mm.append((ti, a + wi, False))
                                    mm.append((ti, 0, True))
                                for i, (ti, w, is_ua) in enumerate(mm):
                                    if is_ua:
                                        nc.tensor.matmul(
                                            dps[:, ti * P:(ti + 1) * P],
                                            lhsT=ident[:],
                                            rhs=uag[:, q0 + ti, :],
                                            start=(i == 0),
                                            stop=(i == len(mm) - 1),
                                            skip_group_check=True)
                                        continue
                                    st = sel_chunk(scur)
                                    ssl = st[:, scur % XCH:scur % XCH + P]
                                    nc.tensor.matmul(
                                        dps[:, ti * P:(ti + 1) * P],
                                        lhsT=ssl,
                                        rhs=ucres[:, w * P:(w + 1) * P],
                                        start=(i == 0), stop=(i == len(mm) - 1),
                                        skip_group_check=True)
                                    scur += P
                                nc.scalar.activation(
                                    rel[:, q0 * P:(q0 + qn) * P],
                                    dps[:, :qn * P],
                                    mybir.ActivationFunctionType.Relu)
                            nc.vector.tensor_mul(
                                rel[:, :cl], rel[:, :cl], w2r_sb[:, :cl])
                            nc.vector.reduce_sum(
                                ysb[:, oc // P:oc // P + ctn],
                                rel[:, :cl].rearrange(
                                    "p (t w) -> p t w", w=P),
                                mybir.AxisListType.X)
                nc.vector.tensor_scalar(
                    out=ysb[:], in0=ysb[:], scalar1=b_sb["b_dec2c"][:],
                    scalar2=None, op0=mybir.AluOpType.add)
                nc.sync.dma_start(out=y_out[:], in_=ysb[:])

    nc.compile()
    return nc


# ---------------------------------------------------------------------------
# entry point
# ---------------------------------------------------------------------------

def _pack_x(xrows, src_order):
    """[etot] indices into xrows [n, 128] -> [128, etot] tile-major pack."""
    g = xrows[src_order]                       # [etot, 128]
    T = g.shape[0] // P
    return np.ascontiguousarray(
        g.reshape(T, P, P).transpose(1, 0, 2).reshape(P, T * P))


def make_in_maps(cfg, inputs, pa1, pc1, pa2, pc2, dec):
    npdt = cfg.npdt
    f = lambda a: np.ascontiguousarray(np.asarray(a), dtype=np.float32)
    xc16 = f(inputs["x_customer"]).astype(npdt)
    xa16 = f(inputs["x_article"]).astype(npdt)
    wd1 = f(inputs["W_dec1"])
    w2 = f(inputs["W_dec2"]).reshape(-1)
    base = dict(
        W_msg1_ca=f(inputs["W_msg1_ca"]).astype(npdt),
        W_self1_a=f(inputs["W_self1_a"]).astype(npdt),
        W_msg1_ac=f(inputs["W_msg1_ac"]).astype(npdt),
        W_self1_c=f(inputs["W_self1_c"]).astype(npdt),
        W_msg2_ca=f(inputs["W_msg2_ca"]).astype(npdt),
        W_self2_a=f(inputs["W_self2_a"]).astype(npdt),
        W_msg2_ac=f(inputs["W_msg2_ac"]).astype(npdt),
        W_self2_c=f(inputs["W_self2_c"]).astype(npdt),
        Wd1c=wd1[:P].astype(npdt), Wd1a=wd1[P:].astype(npdt),
        w2rep=np.tile(w2.astype(npdt).reshape(1, P), (P, DGCH // P)),
        b1_a=f(inputs["b1_a"]).reshape(P, 1),
        b1_c=f(inputs["b1_c"]).reshape(P, 1),
        b2_a=f(inputs["b2_a"]).reshape(P, 1),
        b2_c=f(inputs["b2_c"]).reshape(P, 1),
        bn_gamma_c=f(inputs["bn_gamma_c"]).reshape(P, 1),
        bn_beta_c=f(inputs["bn_beta_c"]).reshape(P, 1),
        bn_gamma_a=f(inputs["bn_gamma_a"]).reshape(P, 1),
        bn_beta_a=f(inputs["bn_beta_a"]).reshape(P, 1),
        b_dec1=f(inputs["b_dec1"]).reshape(P, 1),
        b_dec2c=np.full((P, 1),
                        float(np.asarray(inputs["b_dec2"]).item()),
                        np.float32),
    )
    e_dst = np.asarray(inputs["edge_dst_article"]).astype(np.int64)
    e_srcc = np.asarray(inputs["edge_src_customer"]).astype(np.int64)
    cnt_a = np.bincount(e_dst, minlength=cfg.n_a)
    cnt_c = np.bincount(e_srcc, minlength=cfg.n_c)
    scl_a16 = (1.0 / np.maximum(cnt_a, 1.0)).astype(npdt)
    scl_c16 = (1.0 / np.maximum(cnt_c, 1.0)).astype(npdt)
    in_maps = []
    for k in range(NCORES):
        m = dict(base)
        m["xaT"] = np.ascontiguousarray(
            xa16[k * cfg.apc:(k + 1) * cfg.apc].T)
        m["xcT"] = np.ascontiguousarray(
            xc16[k * cfg.cpc:(k + 1) * cfg.cpc].T)
        m["sclA"] = scl_a16[k * cfg.apc:(k + 1) * cfg.apc].reshape(1, -1)
        m["sclC"] = scl_c16[k * cfg.cpc:(k + 1) * cfg.cpc].reshape(1, -1)
        np8 = cfg.np8
        m["xA1"] = _pack_x(xc16, pa1.src_order[k])
        m["pA1"] = pa1.pstreams[k].astype(np8)
        for h in range(2):
            m[f"xC1{h}"] = _pack_x(xa16, pc1[h].src_order[k])
            m[f"pC1{h}"] = pc1[h].pstreams[k].astype(np8)
            m[f"pC2{h}"] = pc2[h].pstreams[k].astype(np8)
            m[f"iC2{h}"] = pc2[h].idx[k]
        m["pA2"] = pa2.pstreams[k].astype(np8)
        m["iA2"] = pa2.idx[k]
        m["dec_idx_a"] = dec["idx_a"][k]
        m["selD"] = dec["sels"][k].astype(np8)
        in_maps.append(m)
    return in_maps


def run(cfg, inputs, trace=False):
    pa1, pc1, pa2, pc2, dec = prep_all(cfg, inputs)
    in_maps = make_in_maps(cfg, inputs, pa1, pc1, pa2, pc2, dec)
    nc = build_nc(cfg, pa1, pc1, pa2, pc2, dec)
    res = run_bass_kernel_spmd(nc, in_maps, core_ids=list(range(NCORES)),
                               trace=trace)
    y = np.empty(cfg.e_lbl, np.float32)
    for k in range(NCORES):
        yl = res.results[k]["y"].T.reshape(-1)
        po = dec["out_pos"][k]
        vm = po >= 0
        y[po[vm]] = yl[vm]
    return y, res


def kernel(**inputs):
    cfg = Cfg()
    y, _ = run(cfg, inputs, trace=False)
    return y
